# revision 22
# baseline (speedup 1.0000x reference)
"""Trainium2 Bass kernel for nn_Base_55954833932808 (GNN message passing).

Distribution (8 NeuronCores):
  - Data-parallel over graphs: core c owns node rows [2048c, 2048c+2048)
    (graphs [8c, 8c+8)); conv weights replicated.
  - Node-head weights sharded along the node-position axis: core c owns
    positions [32c, 32c+32) for all 64 graphs (expert-parallel).
  - segment_sum(x[src], dst) per dst-shard: host sorts edges by dst, pads
    each 128-node block to T*128 edges; the device gathers x[src] rows
    (dma_gather, bf16, 4 SWDGE queues) and reduces on the TensorEngine with
    per-tile 0/1 selection matrices accumulated in PSUM.
  - Collective overlap: the pre-BN activations h are AllGathered in 4
    chunks as blocks finish (overlapping block compute); BatchNorm+ReLU is
    applied consumer-side next layer using relu(s*h+t) = s*relu(h + t/s)
    (valid: gamma > 0 for this problem), with s folded into the existing
    aggT PSUM->SBUF copy and into the x^T activation pass.
  - x^T tiles for the self-term come from PE transposes of local h (no
    gather); layer 0 uses a host-pretransposed x^T input.
  - The node head's input redistribution is an AllToAll (2 MB/core)
    instead of a full 16 MB AllGather.
"""

import numpy as np
import ml_dtypes

import concourse.bass as bass
import concourse.mybir as mybir
import concourse.tile as tile
from concourse import bacc
from concourse.masks import make_identity
from concourse.bass_utils import run_bass_kernel_spmd

# ---------------- problem constants (hardcoded per contest rules) -------------
NUM_NODES = 256
B = 64                  # graphs
N = NUM_NODES * B       # 16384 nodes
E = N * 16              # 262144 edges
HID = 512
L = 3
DSH = 512
GH1, GH2 = 512, 256
GOUT = 8
NH1, NH2 = 256, 128
EPS = 1e-5

NC = 8                  # cores
NPC = N // NC           # 2048 nodes per core
NB = 16                 # 128-node blocks per core
P = 128
KC = HID // P           # 4 feature chunks
MC = NPC // P           # 16 node chunks per core
POS_PC = NUM_NODES // NC  # 32 node-head positions per core

BF16 = mybir.dt.bfloat16
F32 = mybir.dt.float32
I16 = mybir.dt.int16

_nbf = ml_dtypes.bfloat16


def _wrap_idx(idx, width=None):
    """Wrap an index list into the dma_gather [128, n/16] layout
    (first 16 partitions hold the indices; replicated to all 8 Q7 groups)."""
    n = len(idx)
    assert n % 16 == 0
    w = n // 16
    if width is None:
        width = w
    out = np.zeros((128, width), np.int16)
    blk = np.asarray(idx, np.int16).reshape(w, 16).T  # [16, w]
    for r in range(8):
        out[16 * r:16 * r + 16, :w] = blk
    return out


def _pieces(t):
    """Split t tiles into 4 pieces for the 4 SWDGE queues."""
    q = t // 4
    return [t - 3 * q, q, q, q]


def _hrep_row(n):
    """Node id -> row in the chunk-major replicated-h layout.

    AllGather chunk j (blocks 4j..4j+3 of every core) must land contiguously,
    so hrep rows are ordered [chunk j][core c][block m%4][p]."""
    n = np.asarray(n)
    c = n // NPC
    m = (n % NPC) // P
    p = n % P
    return (m // 4) * (NC * 512) + c * 512 + (m % 4) * P + p


# ------------------------------ host preprocessing ---------------------------

def _host_prep(inputs):
    x = np.asarray(inputs["x"], np.float32)
    ei = np.asarray(inputs["edge_index"], np.int64)
    src, dst = ei[0], ei[1]

    # x rows permuted into the chunk-major hrep layout so one eidx table
    # works for layer 0 (reads x_bf) and layers 1-2 (read hrep)
    x_bf = np.empty((N, HID), _nbf)
    x_bf[_hrep_row(np.arange(N))] = x.astype(_nbf)

    # --- per-core edge structures (node-order ids, no permutation) ---
    order = np.argsort(dst, kind="stable")
    src_s, dst_s = src[order], dst[order]
    core_of = dst_s // NPC

    per_core = []
    counts = np.zeros((NC, NB), np.int64)
    for c in range(NC):
        sel = core_of == c
        s_c, d_c = src_s[sel], dst_s[sel] - c * NPC
        blocks = []
        for b in range(NB):
            bs = d_c // P == b
            blocks.append((s_c[bs], d_c[bs] - b * P))
            counts[c, b] = bs.sum()
        per_core.append(blocks)

    TB = tuple(int(np.ceil(counts[:, b].max() / P)) for b in range(NB))
    OFF = np.concatenate([[0], np.cumsum(TB)])  # tile offsets per block
    TT = int(OFF[-1])

    # dstcol[p, tile] = within-block dst of edge p of that tile (-1 = pad);
    # the device builds the one-hot S tile as (iota == dstcol) on the fly.
    dst_all, idx_all = [], []
    for c in range(NC):
        dstcol = np.full((128, TT), -1.0, np.float32)
        idxw = np.zeros((128, TT * 8), np.int16)
        for b in range(NB):
            s_b, n_b = per_core[c][b]
            cnt = len(s_b)
            TW = TB[b] * P
            pad_idx = np.zeros(TW, np.int64)
            pad_idx[:cnt] = _hrep_row(s_b)  # hrep rows; pads -> row 0
            idxw[:, OFF[b] * 8:OFF[b + 1] * 8] = _wrap_idx(pad_idx, TW // 16)
            tt = np.arange(cnt) // P
            pp = np.arange(cnt) % P
            dstcol[pp, int(OFF[b]) + tt] = n_b
        dst_all.append(dstcol)
        idx_all.append(idxw)

    iota_rep = np.broadcast_to(np.arange(P, dtype=np.float32),
                               (P, P)).astype(_nbf).copy()

    # --- weights ---
    def chunked(w):  # [Kin, F] -> [128, (Kin/128)*F]
        ki, f = w.shape
        return np.ascontiguousarray(
            w.reshape(ki // P, P, f).transpose(1, 0, 2).reshape(P, -1)).astype(_nbf)

    wcat = np.zeros((L, P, 2 * KC * HID), _nbf)
    for l in range(L):
        wself = np.asarray(inputs["conv_wself"][l], np.float32)
        wnei = np.asarray(inputs["conv_wneigh"][l], np.float32)
        wcat[l] = chunked(np.concatenate([wself, wnei], axis=0))

    gs_w1 = chunked(np.asarray(inputs["gs_w1"], np.float32) / NUM_NODES)
    gs_w2 = chunked(np.asarray(inputs["gs_w2"], np.float32))
    gh_w1 = chunked(np.asarray(inputs["gh_w1"], np.float32))
    gh_w2 = chunked(np.asarray(inputs["gh_w2"], np.float32))
    gh_w3 = chunked(np.asarray(inputs["gh_w3"], np.float32))

    def pcol(b):  # [F] -> [128, F/128] f32 (per-partition bias columns)
        return np.ascontiguousarray(
            np.asarray(b, np.float32).reshape(-1, P).T)

    gs_b1 = pcol(inputs["gs_b1"]); gs_b2 = pcol(inputs["gs_b2"])
    gh_b1 = pcol(inputs["gh_b1"]); gh_b2 = pcol(inputs["gh_b2"])
    gh_b3 = np.asarray(inputs["gh_b3"], np.float32).reshape(GOUT, 1)

    pool_ind = np.zeros((P, NB * 8), _nbf)
    for m in range(NB):
        pool_ind[:, m * 8 + m // 2] = 1.0

    bn_g = np.asarray(inputs["bn_gamma"], np.float32)
    bn_b = np.asarray(inputs["bn_beta"], np.float32)

    # --- node-head (per core) ---
    nh_w1 = np.asarray(inputs["nh_w1"], np.float32)   # [256, 512, 256]
    nh_w2 = np.asarray(inputs["nh_w2"], np.float32)   # [256, 256, 128]
    nh_w3 = np.asarray(inputs["nh_w3"], np.float32)   # [256, 128, 1]
    nh_b1 = np.asarray(inputs["nh_b1"], np.float32)
    nh_b2 = np.asarray(inputs["nh_b2"], np.float32)
    nh_b3 = np.asarray(inputs["nh_b3"], np.float32)

    # node-head gather from the AllToAll output buffer:
    # a2a_out row s*256 + pp*8 + g = x3 of (graph 8s+g, position 32c+pp)
    nhidx = np.zeros((128, 16 * 8), np.int16)
    for jj in range(16):
        ids = []
        for half in range(2):
            pp = 2 * jj + half
            for s in range(NC):
                for g in range(8):
                    ids.append(s * 256 + pp * 8 + g)
        nhidx[:, jj * 8:(jj + 1) * 8] = _wrap_idx(ids, 8)

    per_core_maps = []
    for c in range(NC):
        pos = np.arange(POS_PC) + POS_PC * c
        w1 = np.stack([chunked(nh_w1[p]) for p in pos])          # [32,128,4*256]
        w2 = np.stack([chunked(nh_w2[p]) for p in pos])          # [32,128,2*128]
        w3 = np.ascontiguousarray(nh_w3[pos, :, 0].T).astype(_nbf)  # [128, 32]
        b1 = np.zeros((P, POS_PC * 2), np.float32)
        for j in range(POS_PC):
            b1[:, 2 * j] = nh_b1[pos[j], :P]
            b1[:, 2 * j + 1] = nh_b1[pos[j], P:]
        b2 = np.ascontiguousarray(nh_b2[pos].T)                  # [128, 32]
        b3 = nh_b3[pos].reshape(POS_PC, 1)

        # own-shard x^T tiles: xT0[p, k, i] = x[c*NPC + i][k*128 + p]
        xs = x[c * NPC:(c + 1) * NPC]                            # [2048, 512]
        xT0 = np.ascontiguousarray(
            xs.reshape(NPC, KC, P).transpose(2, 1, 0)).astype(_nbf)

        per_core_maps.append(dict(
            x_bf=x_bf, xT0=xT0,
            dstcol=dst_all[c], eidx=idx_all[c], iota_rep=iota_rep,
            wcat=wcat, bn_g=bn_g, bn_b=bn_b,
            gs_w1=gs_w1, gs_w2=gs_w2, gh_w1=gh_w1, gh_w2=gh_w2, gh_w3=gh_w3,
            gs_b1=gs_b1, gs_b2=gs_b2, gh_b1=gh_b1, gh_b2=gh_b2, gh_b3=gh_b3,
            pool_ind=pool_ind,
            nh_w1=w1, nh_w2=w2, nh_w3=w3, nh_b1=b1, nh_b2=b2, nh_b3=b3,
            nh_idx=nhidx,
        ))
    return TB, per_core_maps


# ------------------------------ device program --------------------------------

def _build_program(TB):
    TB = tuple(TB)
    OFF = [0]
    for t in TB:
        OFF.append(OFF[-1] + t)
    TT = OFF[-1]
    TMAX = max(TB)
    PMAX = max(_pieces(TMAX))

    nc = bacc.Bacc("TRN2", target_bir_lowering=False, debug=False,
                   num_devices=NC, num_swdge_queues=4)

    dt = {}
    def din(name, shape, dtype):
        dt[name] = nc.dram_tensor(name, list(shape), dtype, kind="ExternalInput")
        return dt[name]

    din("x_bf", (N, HID), BF16)
    din("xT0", (P, KC, NPC), BF16)
    din("dstcol", (P, TT), F32)
    din("iota_rep", (P, P), BF16)
    din("eidx", (P, TT * 8), I16)
    din("wcat", (L, P, 2 * KC * HID), BF16)
    din("bn_g", (L, HID), F32)
    din("bn_b", (L, HID), F32)
    din("gs_w1", (P, KC * DSH), BF16)
    din("gs_w2", (P, KC * DSH), BF16)
    din("gh_w1", (P, KC * GH1), BF16)
    din("gh_w2", (P, KC * GH2), BF16)
    din("gh_w3", (P, 2 * GOUT), BF16)
    din("gs_b1", (P, KC), F32)
    din("gs_b2", (P, KC), F32)
    din("gh_b1", (P, KC), F32)
    din("gh_b2", (P, GH2 // P), F32)
    din("gh_b3", (GOUT, 1), F32)
    din("pool_ind", (P, NB * 8), BF16)
    din("nh_w1", (POS_PC, P, KC * NH1), BF16)
    din("nh_w2", (POS_PC, P, 2 * NH2), BF16)
    din("nh_w3", (P, POS_PC), BF16)
    din("nh_b1", (P, POS_PC * 2), F32)
    din("nh_b2", (P, POS_PC), F32)
    din("nh_b3", (POS_PC, 1), F32)
    din("nh_idx", (P, 16 * 8), I16)

    out_node = nc.dram_tensor("out_node", [POS_PC, B], F32, kind="ExternalOutput")
    out_graph = nc.dram_tensor("out_graph", [GOUT, B], F32, kind="ExternalOutput")

    # collective buffers (internal DRAM, Shared address space)
    st_in_d, st_out_d = [], []
    ag_in_d, hrep_d = [], []
    for l in range(L):
        st_in_d.append(nc.dram_tensor(f"st_in{l}", [1, 2 * HID], F32))
        st_out_d.append(nc.dram_tensor(f"st_out{l}", [1, 2 * HID], F32,
                                       addr_space="Shared"))
    for l in range(L - 1):
        ag_in_d.append(nc.dram_tensor(f"ag_in{l}", [NPC, HID], BF16))
        hrep_d.append(nc.dram_tensor(f"hrep{l}", [N, HID], BF16,
                                     addr_space="Shared"))
    a2a_in_d = nc.dram_tensor("a2a_in", [NPC, HID], BF16)
    a2a_out_d = nc.dram_tensor("a2a_out", [NPC, HID], BF16)

    groups = [list(range(NC))]
    Relu = mybir.ActivationFunctionType.Relu
    Copy = mybir.ActivationFunctionType.Copy
    Ident = mybir.ActivationFunctionType.Identity
    Square = mybir.ActivationFunctionType.Square
    Sqrt = mybir.ActivationFunctionType.Sqrt
    ADD = mybir.AluOpType.add
    MULT = mybir.AluOpType.mult
    SUB = mybir.AluOpType.subtract

    from contextlib import ExitStack
    with tile.TileContext(nc) as tc, ExitStack() as octx:
        pp = octx.enter_context(tc.tile_pool(name="outer", bufs=1))
        xp = octx.enter_context(tc.tile_pool(name="xnode", bufs=1))
        psA = octx.enter_context(tc.tile_pool(name="psA", bufs=2, space="PSUM"))
        psB = octx.enter_context(tc.tile_pool(name="psB", bufs=2, space="PSUM"))
        psS = octx.enter_context(tc.tile_pool(name="psS", bufs=1, space="PSUM"))
        psT = octx.enter_context(tc.tile_pool(name="psT", bufs=2, space="PSUM"))

        # const APs for activation biases
        zero_c = pp.tile([P, 1], F32, tag="zeroc")
        nc.vector.memset(zero_c[:], 0.0)
        nc.const_aps.aps[(F32, 0.0)] = zero_c[:]
        eps_c = pp.tile([P, 1], F32, tag="epsc")
        nc.vector.memset(eps_c[:], EPS)
        nc.const_aps.aps[(F32, EPS)] = eps_c[:]
        ones_sb = pp.tile([P, 1], BF16, tag="ones")
        nc.vector.memset(ones_sb[:], 1.0)
        ident = pp.tile([P, P], BF16, tag="ident")
        make_identity(nc, ident[:])

        x3 = None

        with ExitStack() as lctx:
            lp = lctx.enter_context(tc.tile_pool(name="lpersist", bufs=1))
            wp = lctx.enter_context(tc.tile_pool(name="wpool", bufs=1))
            htp = lctx.enter_context(tc.tile_pool(name="htp", bufs=2))
            gp = lctx.enter_context(tc.tile_pool(name="gpool", bufs=5))
            g2p = lctx.enter_context(tc.tile_pool(name="g2pool", bufs=4))
            sgp = lctx.enter_context(tc.tile_pool(name="sgen", bufs=4))
            hp = lctx.enter_context(tc.tile_pool(name="hpool", bufs=1))
            wk = lctx.enter_context(tc.tile_pool(name="work", bufs=2))
            sm4 = lctx.enter_context(tc.tile_pool(name="v4", bufs=4))
            sm2 = lctx.enter_context(tc.tile_pool(name="v2", bufs=1))
            smr = lctx.enter_context(tc.tile_pool(name="strep", bufs=2))
            smo = lctx.enter_context(tc.tile_pool(name="stonce", bufs=1))
            spp = lctx.enter_context(tc.tile_pool(name="spart", bufs=2))

            dstc_sb = lp.tile([P, TT], F32, tag="dstc")
            nc.sync.dma_start(dstc_sb[:], dt["dstcol"][:])
            iota_sb = lp.tile([P, P], BF16, tag="iota")
            nc.sync.dma_start(iota_sb[:], dt["iota_rep"][:])
            eidx_sb = lp.tile([P, TT * 8], I16, tag="eidx")
            nc.sync.dma_start(eidx_sb[:], dt["eidx"][:])

            # layer-0 x^T tiles: host-pretransposed
            xT = htp.tile([P, KC, NPC], BF16, tag="hT")
            nc.sync.dma_start(xT[:], dt["xT0"][:])

            s_part = t_part = tp_rep = None   # BN params of previous layer

            for l in range(L):
                gsrc = dt["x_bf"][:] if l == 0 else hrep_d[l - 1][:]

                w_sb = wp.tile([P, 2 * KC, HID], BF16, tag="wcat")
                nc.sync.dma_start(
                    w_sb[:], dt["wcat"][l].rearrange("p (k f) -> p k f",
                                                     k=2 * KC))

                h_sb = hp.tile([P, MC, HID], BF16, tag="h")
                if l < L - 1:
                    hTn = htp.tile([P, KC, NPC], BF16, tag="hT")
                else:
                    hTn = None

                st_sum = psS.tile([1, HID], F32, space="PSUM", tag="stsum")
                st_sq = psS.tile([1, HID], F32, space="PSUM", tag="stsq")

                for m in range(NB):
                    T_b = TB[m]
                    agg_ps = psA.tile([P, HID], F32, space="PSUM", tag="agg")
                    t0 = 0
                    tglob = 0
                    for piece, tcnt in enumerate(_pieces(T_b)):
                        if tcnt == 0:
                            continue
                        G = gp.tile([P, PMAX, HID], BF16, tag="G")
                        nc.gpsimd.dma_gather(
                            G[:, :tcnt, :], gsrc,
                            eidx_sb[:, (OFF[m] + t0) * 8:(OFF[m] + t0 + tcnt) * 8],
                            tcnt * P, tcnt * P, HID, single_packet=False,
                            queue_num=piece)
                        for t in range(tcnt):
                            if l == 0:
                                rhs = G[:, t, :]
                            else:
                                G2 = g2p.tile([P, HID], BF16, tag="G2")
                                nc.vector.tensor_tensor(
                                    G2[:], G[:, t, :], tp_rep[:], ADD)
                                nc.vector.tensor_scalar_max(G2[:], G2[:], 0.0)
                                rhs = G2[:]
                            S_t = sgp.tile([P, P], BF16, tag="St")
                            nc.vector.tensor_scalar(
                                S_t[:], iota_sb[:],
                                dstc_sb[:, OFF[m] + t0 + t:OFF[m] + t0 + t + 1],
                                None, mybir.AluOpType.is_equal)
                            nc.tensor.matmul(
                                agg_ps[:], S_t[:],
                                rhs, start=(tglob == 0),
                                stop=(tglob == T_b - 1))
                            tglob += 1
                        t0 += tcnt
                    agg_sb = wk.tile([P, HID], BF16, tag="aggsb")
                    nc.scalar.activation(agg_sb[:], agg_ps[:], Copy)

                    # transpose agg (PE, via identity); fold in BN scale s
                    aggT = wk.tile([P, KC, P], BF16, tag="aggT")
                    for k in range(KC):
                        tr_ps = psT.tile([P, P], BF16, space="PSUM", tag="tr")
                        nc.tensor.transpose(tr_ps[:],
                                            agg_sb[:, k * P:(k + 1) * P],
                                            ident[:])
                        if l == 0:
                            nc.scalar.activation(aggT[:, k, :], tr_ps[:], Copy)
                        else:
                            nc.scalar.activation(aggT[:, k, :], tr_ps[:], Copy,
                                                 scale=s_part[:, k:k + 1])

                    # conv: h[m] = [x|agg] @ wcat
                    h_ps = psB.tile([P, HID], F32, space="PSUM", tag="conv")
                    for k in range(KC):
                        nc.tensor.matmul(h_ps[:], xT[:, k, m * P:(m + 1) * P],
                                         w_sb[:, k, :], start=(k == 0),
                                         stop=False)
                    for k in range(KC):
                        nc.tensor.matmul(h_ps[:], aggT[:, k, :],
                                         w_sb[:, KC + k, :], start=False,
                                         stop=(k == KC - 1))
                    nc.vector.tensor_copy(h_sb[:, m, :], h_ps[:])

                    hsq = wk.tile([P, HID], BF16, tag="hsq")
                    nc.scalar.activation(hsq[:], h_ps[:], Square)
                    nc.tensor.matmul(st_sum[:], ones_sb[:], h_sb[:, m, :],
                                     start=(m == 0), stop=(m == NB - 1))
                    nc.tensor.matmul(st_sq[:], ones_sb[:], hsq[:],
                                     start=(m == 0), stop=(m == NB - 1))

                    # h^T tiles for next layer's self term (PE transpose)
                    if hTn is not None:
                        for k in range(KC):
                            tr2 = psT.tile([P, P], BF16, space="PSUM", tag="tr")
                            nc.tensor.transpose(
                                tr2[:], h_sb[:, m, k * P:(k + 1) * P], ident[:])
                            nc.scalar.activation(
                                hTn[:, k, m * P:(m + 1) * P], tr2[:], Copy)

                    # chunked AllGather of pre-BN h (overlaps block compute)
                    if l < L - 1 and m % 4 == 3:
                        j = m // 4
                        nc.sync.dma_start(
                            ag_in_d[l][512 * j:512 * (j + 1), :].rearrange(
                                "(m p) f -> p m f", p=P),
                            h_sb[:, m - 3:m + 1, :])
                        nc.gpsimd.collective_compute(
                            "AllGather", mybir.AluOpType.bypass,
                            replica_groups=groups,
                            ins=[ag_in_d[l][512 * j:512 * (j + 1), :].opt()],
                            outs=[hrep_d[l][NC * 512 * j:
                                            NC * 512 * (j + 1), :].opt()])

                # ---- BN stats allreduce ----
                stat_sb = sm2.tile([1, 2 * HID], F32, tag="v2h")
                nc.vector.tensor_copy(stat_sb[:, :HID], st_sum[:])
                nc.vector.tensor_copy(stat_sb[:, HID:], st_sq[:])
                nc.sync.dma_start(st_in_d[l][:], stat_sb[:])
                nc.gpsimd.collective_compute(
                    "AllReduce", ADD, replica_groups=groups,
                    ins=[st_in_d[l][:].opt()], outs=[st_out_d[l][:].opt()])
                stat_r = sm2.tile([1, 2 * HID], F32, tag="v2h")
                nc.sync.dma_start(stat_r[:], st_out_d[l][:])

                # s = gamma / sqrt(var+eps); t = beta - mean*s; tp = t/s
                mean = sm4.tile([1, HID], F32, tag="v1h")
                nc.vector.tensor_scalar_mul(mean[:], stat_r[:, :HID], 1.0 / N)
                m2t = sm4.tile([1, HID], F32, tag="v1h")
                nc.vector.tensor_tensor(m2t[:], mean[:], mean[:], MULT)
                var = sm4.tile([1, HID], F32, tag="v1h")
                nc.vector.tensor_scalar(var[:], stat_r[:, HID:], 1.0 / N, None,
                                        MULT)
                nc.vector.tensor_tensor(var[:], var[:], m2t[:], SUB)
                std = sm4.tile([1, HID], F32, tag="v1h")
                nc.scalar.activation(std[:], var[:], Sqrt, bias=EPS)
                inv = sm4.tile([1, HID], F32, tag="v1h")
                nc.vector.reciprocal(inv[:], std[:])
                gam = sm4.tile([1, HID], F32, tag="v1h")
                nc.sync.dma_start(gam[:], dt["bn_g"][l][None, :])
                st_pack = sm2.tile([1, 3 * HID], F32, tag="v3h")
                nc.vector.tensor_tensor(st_pack[:, :HID], gam[:], inv[:], MULT)
                ms = sm4.tile([1, HID], F32, tag="v1h")
                nc.vector.tensor_tensor(ms[:], mean[:], st_pack[:, :HID], MULT)
                bet = sm4.tile([1, HID], F32, tag="v1h")
                nc.sync.dma_start(bet[:], dt["bn_b"][l][None, :])
                nc.vector.tensor_tensor(st_pack[:, HID:2 * HID], bet[:], ms[:],
                                        SUB)
                # tp = t / s  (= beta/s - mean; s > 0 since gamma > 0)
                sinv = sm4.tile([1, HID], F32, tag="v1h")
                nc.vector.reciprocal(sinv[:], st_pack[:, :HID])
                nc.vector.tensor_tensor(st_pack[:, 2 * HID:],
                                        st_pack[:, HID:2 * HID], sinv[:], MULT)
                st_dram = nc.dram_tensor(f"st_pack{l}", [1, 3 * HID], F32)
                nc.sync.dma_start(st_dram[:], st_pack[:])

                # per-partition [128, KC] views of s and t
                s_part = spp.tile([P, KC], F32, tag="sp")
                nc.sync.dma_start(
                    s_part[:],
                    st_dram[0:1, :HID].rearrange("o (k p) -> (o p) k", p=P))
                t_part = spp.tile([P, KC], F32, tag="tp")
                nc.sync.dma_start(
                    t_part[:],
                    st_dram[0:1, HID:2 * HID].rearrange("o (k p) -> (o p) k",
                                                        p=P))
                # replicated t/s row for consumer-side G bias
                tp_rep = smr.tile([P, HID], F32, tag="tprep")
                nc.sync.dma_start(
                    tp_rep[:],
                    st_dram[0:1, 2 * HID:].to_broadcast((P, HID)))

                if l < L - 1:
                    # turn h^T into x^T for next layer: relu(s*h + t),
                    # in place, per-partition scale/bias
                    for k in range(KC):
                        nc.scalar.activation(hTn[:, k, :], hTn[:, k, :], Relu,
                                             scale=s_part[:, k:k + 1],
                                             bias=t_part[:, k:k + 1])
                    xT = hTn
                else:
                    # final layer: normalize local x3 = relu(s*h + t)
                    s_rep = smo.tile([P, HID], F32, tag="srep")
                    nc.sync.dma_start(
                        s_rep[:], st_dram[0:1, :HID].to_broadcast((P, HID)))
                    t_rep = smo.tile([P, HID], F32, tag="trep")
                    nc.sync.dma_start(
                        t_rep[:],
                        st_dram[0:1, HID:2 * HID].to_broadcast((P, HID)))
                    x3 = xp.tile([P, MC, HID], BF16, tag="xnode")
                    for m in range(NB):
                        tmp = wk.tile([P, HID], F32, tag="norm")
                        nc.vector.tensor_tensor(tmp[:], h_sb[:, m, :],
                                                s_rep[:], MULT)
                        nc.vector.tensor_tensor(tmp[:], tmp[:], t_rep[:], ADD)
                        nc.scalar.activation(x3[:, m, :], tmp[:], Relu)

            # ---- AllToAll: redistribute x3 by node position ----
            x3v = x3[:].rearrange("p (g two) f -> p two g f", two=2)
            for d in range(NC):
                nc.sync.dma_start(
                    a2a_in_d[d * 256:(d + 1) * 256, :].rearrange(
                        "(pp g) f -> pp g f", g=8),
                    x3v[32 * (d % 4):32 * (d % 4) + 32, d // 4, :, :])
            nc.gpsimd.collective_compute(
                "AllToAll", mybir.AluOpType.bypass, replica_groups=groups,
                ins=[a2a_in_d[:].opt()], outs=[a2a_out_d[:].opt()])

        with ExitStack() as hctx:
            sm = hctx.enter_context(tc.tile_pool(name="once", bufs=1))
            wk2 = hctx.enter_context(tc.tile_pool(name="hwork", bufs=3))

            # ------------- graph path (local graphs) -------------
            pind = sm.tile([P, NB, 8], BF16, tag="pind")
            nc.sync.dma_start(
                pind[:],
                dt["pool_ind"][:].rearrange("p (m g) -> p m g", g=8))
            xgT = sm.tile([P, KC, 8], BF16, tag="xgT")
            for k in range(KC):
                pool_ps = psA.tile([P, HID], F32, space="PSUM", tag="agg")
                for m in range(NB):
                    nc.tensor.matmul(pool_ps[:, :8],
                                     x3[:, m, k * P:(k + 1) * P],
                                     pind[:, m, :], start=(m == 0),
                                     stop=(m == NB - 1))
                nc.scalar.activation(xgT[:, k, :], pool_ps[:, :8], Relu)

            def mlp_layer(src, w_dram, b_dram, kin, kout, act, tag):
                w_sb2 = sm.tile([P, kin * (kout * P)], BF16, tag=tag + "w")
                nc.sync.dma_start(w_sb2[:], w_dram[:])
                w3v = w_sb2[:].rearrange("p (k f) -> p k f", k=kin)
                b_sb = sm.tile([P, kout], F32, tag=tag + "b")
                nc.sync.dma_start(b_sb[:], b_dram[:])
                res = sm.tile([P, kout, 8], BF16, tag=tag + "o")
                for mo in range(kout):
                    ps = psB.tile([P, HID], F32, space="PSUM", tag="conv")
                    for k in range(kin):
                        nc.tensor.matmul(ps[:, :8],
                                         w3v[:, k, mo * P:(mo + 1) * P],
                                         src[:, k, :], start=(k == 0),
                                         stop=(k == kin - 1))
                    nc.scalar.activation(res[:, mo, :], ps[:, :8], act,
                                         bias=b_sb[:, mo:mo + 1])
                return res

            z1 = mlp_layer(xgT, dt["gs_w1"], dt["gs_b1"], KC, KC, Ident,
                           "gsw1")
            z2 = mlp_layer(z1, dt["gs_w2"], dt["gs_b2"], KC, KC, Relu,
                           "gsw2")
            g1 = mlp_layer(z2, dt["gh_w1"], dt["gh_b1"], KC, KC, Relu,
                           "ghw1")
            g2 = mlp_layer(g1, dt["gh_w2"], dt["gh_b2"], KC, GH2 // P,
                           Relu, "ghw2")
            w3_sb = sm.tile([P, 2 * GOUT], BF16, tag="ghw3")
            nc.sync.dma_start(w3_sb[:], dt["gh_w3"][:])
            b3_sb = sm.tile([GOUT, 1], F32, tag="ghb3")
            nc.sync.dma_start(b3_sb[:], dt["gh_b3"][:])
            g_ps = psB.tile([P, HID], F32, space="PSUM", tag="conv")
            for k in range(2):
                nc.tensor.matmul(g_ps[:GOUT, :8],
                                 w3_sb[:, k * GOUT:(k + 1) * GOUT],
                                 g2[:, k, :], start=(k == 0),
                                 stop=(k == 1))
            gsb = sm.tile([GOUT, B], F32, tag="gsb")
            nc.vector.memset(gsb[:], 0.0)
            nc.scalar.activation(gsb[:, :8], g_ps[:GOUT, :8], Ident,
                                 bias=b3_sb[:])
            nc.sync.dma_start(out_graph[:], gsb[:])

            # ------------- node head -------------
            nhw3_sb = sm.tile([P, POS_PC], BF16, tag="nhw3")
            nc.sync.dma_start(nhw3_sb[:], dt["nh_w3"][:])
            nhb1_sb = sm.tile([P, POS_PC * 2], F32, tag="nhb1")
            nc.sync.dma_start(nhb1_sb[:], dt["nh_b1"][:])
            nhb2_sb = sm.tile([P, POS_PC], F32, tag="nhb2")
            nc.sync.dma_start(nhb2_sb[:], dt["nh_b2"][:])
            nhb3_sb = sm.tile([POS_PC, 1], F32, tag="nhb3")
            nc.sync.dma_start(nhb3_sb[:], dt["nh_b3"][:])
            nhidx_sb = sm.tile([P, 16 * 8], I16, tag="nhidx")
            nc.sync.dma_start(nhidx_sb[:], dt["nh_idx"][:])

            nodeflat = sm.tile([1, POS_PC * B], F32, tag="nodeflat")
            for jj in range(16):
                xpT = wk2.tile([P, KC, P], BF16, tag="xpT")
                nc.gpsimd.dma_gather(xpT[:], a2a_out_d[:],
                                     nhidx_sb[:, jj * 8:(jj + 1) * 8],
                                     P, P, HID, transpose=True,
                                     queue_num=jj % 4)
                for half in range(2):
                    j = 2 * jj + half
                    w1_sb = wk2.tile([P, KC, NH1], BF16, tag="nhw1")
                    nc.sync.dma_start(
                        w1_sb[:],
                        dt["nh_w1"][j].rearrange("p (k f) -> p k f", k=KC))
                    w2_sb = wk2.tile([P, 2, NH2], BF16, tag="nhw2")
                    nc.sync.dma_start(
                        w2_sb[:],
                        dt["nh_w2"][j].rearrange("p (k f) -> p k f", k=2))
                    rhs = xpT[:, :, half * B:(half + 1) * B]
                    h1T = wk2.tile([P, 2, B], BF16, tag="h1T")
                    for mo in range(2):
                        ps = psB.tile([P, HID], F32, space="PSUM",
                                      tag="conv")
                        for k in range(KC):
                            nc.tensor.matmul(
                                ps[:, :B],
                                w1_sb[:, k, mo * P:(mo + 1) * P],
                                rhs[:, k, :], start=(k == 0),
                                stop=(k == KC - 1))
                        nc.scalar.activation(
                            h1T[:, mo, :], ps[:, :B], Relu,
                            bias=nhb1_sb[:, 2 * j + mo:2 * j + mo + 1])
                    ps2 = psB.tile([P, HID], F32, space="PSUM", tag="conv")
                    for k in range(2):
                        nc.tensor.matmul(ps2[:, :B], w2_sb[:, k, :],
                                         h1T[:, k, :], start=(k == 0),
                                         stop=(k == 1))
                    h2T = wk2.tile([P, B], BF16, tag="h2T")
                    nc.scalar.activation(h2T[:], ps2[:, :B], Relu,
                                         bias=nhb2_sb[:, j:j + 1])
                    ps3 = psA.tile([P, HID], F32, space="PSUM", tag="agg")
                    nc.tensor.matmul(ps3[:1, :B], nhw3_sb[:, j:j + 1],
                                     h2T[:], start=True, stop=True)
                    nc.scalar.activation(
                        nodeflat[0:1, j * B:(j + 1) * B], ps3[:1, :B],
                        Copy)
            nflat_d = nc.dram_tensor("nflat_d", [1, POS_PC * B], F32)
            nc.sync.dma_start(nflat_d[:], nodeflat[:])
            nodeT = sm.tile([POS_PC, B], F32, tag="nodeT")
            nc.sync.dma_start(
                nodeT[:],
                nflat_d[:].rearrange("o (j g) -> (o j) g", j=POS_PC))
            nodeS = sm.tile([POS_PC, B], F32, tag="nodeS")
            nc.vector.tensor_tensor(
                nodeS[:], nodeT[:],
                nhb3_sb[:].to_broadcast((POS_PC, B)), ADD)
            nc.sync.dma_start(out_node[:], nodeS[:])

    nc.compile()
    return nc


_PROG_CACHE = {}


def _get_program(TB):
    key = tuple(TB)
    if key not in _PROG_CACHE:
        _PROG_CACHE[key] = _build_program(TB)
    return _PROG_CACHE[key]


def kernel(**inputs):
    res = _run(inputs)
    return _assemble(res)


_LAST_RES = None


def _run(inputs, debug=False, trace=False):
    global _LAST_RES
    TB, maps = _host_prep(inputs)
    nc = _get_program(TB)
    res = run_bass_kernel_spmd(nc, maps, list(range(NC)), trace=trace)
    _LAST_RES = res
    if trace:
        print(f"HW exec time: {res.exec_time_ns} ns")
        print(f"mean exec time: {res.mean_exec_time_ns} ns "
              f"(max core {res.max_exec_time_core_id})")
    return res.results


def _assemble(results):
    full = np.empty((B, GOUT + NUM_NODES), np.float32)
    for c in range(NC):
        full[:, GOUT + POS_PC * c: GOUT + POS_PC * (c + 1)] = \
            results[c]["out_node"].T
        full[8 * c:8 * (c + 1), :GOUT] = results[c]["out_graph"][:, :8].T
    return full


# revision 33
# speedup vs baseline: 1.1915x; 1.1915x over previous
"""Trainium2 Bass kernel for nn_Base_55954833932808 (GNN message passing).

Distribution (8 NeuronCores):
  - Data-parallel over graphs: core c owns node rows [2048c, 2048c+2048)
    (graphs [8c, 8c+8)); conv weights replicated.
  - Node-head weights sharded along the node-position axis: core c owns
    positions [32c, 32c+32) for all 64 graphs (expert-parallel).
  - segment_sum(x[src], dst) per dst-shard: host sorts edges by dst, pads
    each 128-node block to T*128 edges; the device gathers x[src] rows
    (dma_gather, bf16, 4 SWDGE queues) and reduces on the TensorEngine with
    per-tile 0/1 selection matrices accumulated in PSUM.
  - Collective overlap: the pre-BN activations h are AllGathered in 4
    chunks as blocks finish (overlapping block compute); BatchNorm+ReLU is
    applied consumer-side next layer using relu(s*h+t) = s*relu(h + t/s)
    (valid: gamma > 0 for this problem), with s folded into the existing
    aggT PSUM->SBUF copy and into the x^T activation pass.
  - x^T tiles for the self-term come from PE transposes of local h (no
    gather); layer 0 uses a host-pretransposed x^T input.
  - The node head's input redistribution is an AllToAll (2 MB/core)
    instead of a full 16 MB AllGather.
"""

import numpy as np
import ml_dtypes

import concourse.bass as bass
import concourse.mybir as mybir
import concourse.tile as tile
from concourse import bacc
from concourse.masks import make_identity
from concourse.bass_utils import run_bass_kernel_spmd

# ---------------- problem constants (hardcoded per contest rules) -------------
NUM_NODES = 256
B = 64                  # graphs
N = NUM_NODES * B       # 16384 nodes
E = N * 16              # 262144 edges
HID = 512
L = 3
DSH = 512
GH1, GH2 = 512, 256
GOUT = 8
NH1, NH2 = 256, 128
EPS = 1e-5

NC = 8                  # cores
NPC = N // NC           # 2048 nodes per core
NB = 16                 # 128-node blocks per core
P = 128
KC = HID // P           # 4 feature chunks
MC = NPC // P           # 16 node chunks per core
POS_PC = NUM_NODES // NC  # 32 node-head positions per core

BF16 = mybir.dt.bfloat16
F32 = mybir.dt.float32
I16 = mybir.dt.int16

_nbf = ml_dtypes.bfloat16


def _wrap_idx(idx, width=None):
    """Wrap an index list into the dma_gather [128, n/16] layout
    (first 16 partitions hold the indices; replicated to all 8 Q7 groups)."""
    n = len(idx)
    assert n % 16 == 0
    w = n // 16
    if width is None:
        width = w
    out = np.zeros((128, width), np.int16)
    blk = np.asarray(idx, np.int16).reshape(w, 16).T  # [16, w]
    for r in range(8):
        out[16 * r:16 * r + 16, :w] = blk
    return out


def _pieces(t):
    """Split t tiles into 4 pieces for the 4 SWDGE queues."""
    q = t // 4
    return [t - 3 * q, q, q, q]


def _hrep_row(n):
    """Node id -> row in the chunk-major replicated-h layout.

    AllGather chunk j (blocks 4j..4j+3 of every core) must land contiguously,
    so hrep rows are ordered [chunk j][core c][block m%4][p]."""
    n = np.asarray(n)
    c = n // NPC
    m = (n % NPC) // P
    p = n % P
    return (m // 4) * (NC * 512) + c * 512 + (m % 4) * P + p


# ------------------------------ host preprocessing ---------------------------

def _host_prep(inputs):
    x = np.asarray(inputs["x"], np.float32)
    ei = np.asarray(inputs["edge_index"], np.int64)
    src, dst = ei[0], ei[1]

    # x rows permuted into the chunk-major hrep layout so one eidx table
    # works for layer 0 (reads x_bf) and layers 1-2 (read hrep)
    x_bf = np.empty((N, HID), _nbf)
    x_bf[_hrep_row(np.arange(N))] = x.astype(_nbf)

    # --- per-core edge structures (node-order ids, no permutation) ---
    order = np.argsort(dst, kind="stable")
    src_s, dst_s = src[order], dst[order]
    core_of = dst_s // NPC

    per_core = []
    counts = np.zeros((NC, NB), np.int64)
    for c in range(NC):
        sel = core_of == c
        s_c, d_c = src_s[sel], dst_s[sel] - c * NPC
        blocks = []
        for b in range(NB):
            bs = d_c // P == b
            blocks.append((s_c[bs], d_c[bs] - b * P))
            counts[c, b] = bs.sum()
        per_core.append(blocks)

    TB = tuple(int(np.ceil(counts[:, b].max() / P)) for b in range(NB))
    OFF = np.concatenate([[0], np.cumsum(TB)])  # tile offsets per block
    TT = int(OFF[-1])

    # dstcol[p, tile] = within-block dst of edge p of that tile (-1 = pad);
    # the device builds the one-hot S tile as (iota == dstcol) on the fly.
    # Edges within a block are sorted by source AG chunk so that gather
    # pieces referencing only early chunks need not wait for the last
    # AllGather chunk of the previous layer.
    dst_all, idx_all = [], []
    # tile_maxchunk[b][t] = max source chunk referenced by tile t of block b
    tile_maxchunk = [[0] * TB[b] for b in range(NB)]
    for c in range(NC):
        dstcol = np.full((128, TT), -1.0, np.float32)
        idxw = np.zeros((128, TT * 8), np.int16)
        for b in range(NB):
            s_b, n_b = per_core[c][b]
            rows = _hrep_row(s_b)
            chunk = rows // (NC * 512)
            o2 = np.argsort(chunk, kind="stable")
            rows, n_b, chunk = rows[o2], n_b[o2], chunk[o2]
            cnt = len(rows)
            TW = TB[b] * P
            pad_idx = np.zeros(TW, np.int64)
            pad_idx[:cnt] = rows            # pads -> row 0 (chunk 0)
            idxw[:, OFF[b] * 8:OFF[b + 1] * 8] = _wrap_idx(pad_idx, TW // 16)
            tt = np.arange(cnt) // P
            pp = np.arange(cnt) % P
            dstcol[pp, int(OFF[b]) + tt] = n_b
            for t in range(TB[b]):
                ch = chunk[t * P:(t + 1) * P]
                mx = int(ch.max()) if len(ch) else 0
                tile_maxchunk[b][t] = max(tile_maxchunk[b][t], mx)
        dst_all.append(dstcol)
        idx_all.append(idxw)

    iota_rep = np.broadcast_to(np.arange(P, dtype=np.float32),
                               (P, P)).astype(_nbf).copy()

    # --- weights ---
    def chunked(w):  # [Kin, F] -> [128, (Kin/128)*F]
        ki, f = w.shape
        return np.ascontiguousarray(
            w.reshape(ki // P, P, f).transpose(1, 0, 2).reshape(P, -1)).astype(_nbf)

    wcat = np.zeros((L, P, 2 * KC * HID), _nbf)
    for l in range(L):
        wself = np.asarray(inputs["conv_wself"][l], np.float32)
        wnei = np.asarray(inputs["conv_wneigh"][l], np.float32)
        wcat[l] = chunked(np.concatenate([wself, wnei], axis=0))

    gs_w1 = chunked(np.asarray(inputs["gs_w1"], np.float32) / NUM_NODES)
    gs_w2 = chunked(np.asarray(inputs["gs_w2"], np.float32))
    gh_w1 = chunked(np.asarray(inputs["gh_w1"], np.float32))
    gh_w2 = chunked(np.asarray(inputs["gh_w2"], np.float32))
    gh_w3 = chunked(np.asarray(inputs["gh_w3"], np.float32))

    def pcol(b):  # [F] -> [128, F/128] f32 (per-partition bias columns)
        return np.ascontiguousarray(
            np.asarray(b, np.float32).reshape(-1, P).T)

    gs_b1 = pcol(inputs["gs_b1"]); gs_b2 = pcol(inputs["gs_b2"])
    gh_b1 = pcol(inputs["gh_b1"]); gh_b2 = pcol(inputs["gh_b2"])
    gh_b3 = np.asarray(inputs["gh_b3"], np.float32).reshape(GOUT, 1)

    pool_ind = np.zeros((P, NB * 8), _nbf)
    for m in range(NB):
        pool_ind[:, m * 8 + m // 2] = 1.0

    bn_g = np.asarray(inputs["bn_gamma"], np.float32)
    bn_b = np.asarray(inputs["bn_beta"], np.float32)

    # --- node-head (per core) ---
    nh_w1 = np.asarray(inputs["nh_w1"], np.float32)   # [256, 512, 256]
    nh_w2 = np.asarray(inputs["nh_w2"], np.float32)   # [256, 256, 128]
    nh_w3 = np.asarray(inputs["nh_w3"], np.float32)   # [256, 128, 1]
    nh_b1 = np.asarray(inputs["nh_b1"], np.float32)
    nh_b2 = np.asarray(inputs["nh_b2"], np.float32)
    nh_b3 = np.asarray(inputs["nh_b3"], np.float32)

    # node-head gather from the AllToAll output buffer:
    # a2a_out row s*256 + pp*8 + g = x3 of (graph 8s+g, position 32c+pp)
    nhidx = np.zeros((128, 16 * 8), np.int16)
    for jj in range(16):
        ids = []
        for half in range(2):
            pp = 2 * jj + half
            for s in range(NC):
                for g in range(8):
                    ids.append(s * 256 + pp * 8 + g)
        nhidx[:, jj * 8:(jj + 1) * 8] = _wrap_idx(ids, 8)

    per_core_maps = []
    for c in range(NC):
        pos = np.arange(POS_PC) + POS_PC * c
        w1 = np.stack([chunked(nh_w1[p]) for p in pos])          # [32,128,4*256]
        w2 = np.stack([chunked(nh_w2[p]) for p in pos])          # [32,128,2*128]
        w3 = np.ascontiguousarray(nh_w3[pos, :, 0].T).astype(_nbf)  # [128, 32]
        b1 = np.zeros((P, POS_PC * 2), np.float32)
        for j in range(POS_PC):
            b1[:, 2 * j] = nh_b1[pos[j], :P]
            b1[:, 2 * j + 1] = nh_b1[pos[j], P:]
        b2 = np.ascontiguousarray(nh_b2[pos].T)                  # [128, 32]
        b3 = nh_b3[pos].reshape(POS_PC, 1)

        # own-shard x^T tiles: xT0[p, k, i] = x[c*NPC + i][k*128 + p]
        xs = x[c * NPC:(c + 1) * NPC]                            # [2048, 512]
        xT0 = np.ascontiguousarray(
            xs.reshape(NPC, KC, P).transpose(2, 1, 0)).astype(_nbf)

        per_core_maps.append(dict(
            x_bf=x_bf, xT0=xT0,
            dstcol=dst_all[c].astype(_nbf), eidx=idx_all[c],
            iota_rep=iota_rep,
            wcat=wcat, bn_g=bn_g, bn_b=bn_b,
            gs_w1=gs_w1, gs_w2=gs_w2, gh_w1=gh_w1, gh_w2=gh_w2, gh_w3=gh_w3,
            gs_b1=gs_b1, gs_b2=gs_b2, gh_b1=gh_b1, gh_b2=gh_b2, gh_b3=gh_b3,
            pool_ind=pool_ind,
            nh_w1=w1, nh_w2=w2, nh_w3=w3, nh_b1=b1, nh_b2=b2, nh_b3=b3,
            nh_idx=nhidx,
        ))
    return (TB, tuple(tuple(t) for t in tile_maxchunk)), per_core_maps


# ------------------------------ device program --------------------------------

def _build_program(key):
    TB, tile_maxchunk = key
    TB = tuple(TB)
    OFF = [0]
    for t in TB:
        OFF.append(OFF[-1] + t)
    TT = OFF[-1]
    TMAX = max(TB)
    PMAX = max(_pieces(TMAX))

    nc = bacc.Bacc("TRN2", target_bir_lowering=False, debug=False,
                   num_devices=NC, num_swdge_queues=4)

    dt = {}
    def din(name, shape, dtype):
        dt[name] = nc.dram_tensor(name, list(shape), dtype, kind="ExternalInput")
        return dt[name]

    din("x_bf", (N, HID), BF16)
    din("xT0", (P, KC, NPC), BF16)
    din("dstcol", (P, TT), BF16)
    din("iota_rep", (P, P), BF16)
    din("eidx", (P, TT * 8), I16)
    din("wcat", (L, P, 2 * KC * HID), BF16)
    din("bn_g", (L, HID), F32)
    din("bn_b", (L, HID), F32)
    din("gs_w1", (P, KC * DSH), BF16)
    din("gs_w2", (P, KC * DSH), BF16)
    din("gh_w1", (P, KC * GH1), BF16)
    din("gh_w2", (P, KC * GH2), BF16)
    din("gh_w3", (P, 2 * GOUT), BF16)
    din("gs_b1", (P, KC), F32)
    din("gs_b2", (P, KC), F32)
    din("gh_b1", (P, KC), F32)
    din("gh_b2", (P, GH2 // P), F32)
    din("gh_b3", (GOUT, 1), F32)
    din("pool_ind", (P, NB * 8), BF16)
    din("nh_w1", (POS_PC, P, KC * NH1), BF16)
    din("nh_w2", (POS_PC, P, 2 * NH2), BF16)
    din("nh_w3", (P, POS_PC), BF16)
    din("nh_b1", (P, POS_PC * 2), F32)
    din("nh_b2", (P, POS_PC), F32)
    din("nh_b3", (POS_PC, 1), F32)
    din("nh_idx", (P, 16 * 8), I16)

    out_node = nc.dram_tensor("out_node", [POS_PC, B], F32, kind="ExternalOutput")
    out_graph = nc.dram_tensor("out_graph", [GOUT, B], F32, kind="ExternalOutput")

    # collective buffers (internal DRAM, Shared address space)
    st_in_d, st_out_d = [], []
    ag_in_d, hrep_d = [], []
    for l in range(L):
        st_in_d.append(nc.dram_tensor(f"st_in{l}", [1, 2 * HID], F32))
        st_out_d.append(nc.dram_tensor(f"st_out{l}", [1, 2 * HID], F32,
                                       addr_space="Shared"))
    for l in range(L - 1):
        ag_in_d.append(nc.dram_tensor(f"ag_in{l}", [NPC, HID], BF16))
        hrep_d.append(nc.dram_tensor(f"hrep{l}", [N, HID], BF16,
                                     addr_space="Shared"))
    a2a_in_d = nc.dram_tensor("a2a_in", [NPC, HID], BF16)
    a2a_out_d = nc.dram_tensor("a2a_out", [NPC, HID], BF16)

    groups = [list(range(NC))]
    Relu = mybir.ActivationFunctionType.Relu
    Copy = mybir.ActivationFunctionType.Copy
    Ident = mybir.ActivationFunctionType.Identity
    Square = mybir.ActivationFunctionType.Square
    Sqrt = mybir.ActivationFunctionType.Sqrt
    ADD = mybir.AluOpType.add
    MULT = mybir.AluOpType.mult
    SUB = mybir.AluOpType.subtract

    from contextlib import ExitStack
    with tile.TileContext(nc) as tc, ExitStack() as octx:
        pp = octx.enter_context(tc.tile_pool(name="outer", bufs=1))
        xp = octx.enter_context(tc.tile_pool(name="xnode", bufs=1))
        psA = octx.enter_context(tc.tile_pool(name="psA", bufs=2, space="PSUM"))
        psB = octx.enter_context(tc.tile_pool(name="psB", bufs=2, space="PSUM"))
        psS = octx.enter_context(tc.tile_pool(name="psS", bufs=1, space="PSUM"))
        psT = octx.enter_context(tc.tile_pool(name="psT", bufs=2, space="PSUM"))

        # const APs for activation biases
        zero_c = pp.tile([P, 1], F32, tag="zeroc")
        nc.vector.memset(zero_c[:], 0.0)
        nc.const_aps.aps[(F32, 0.0)] = zero_c[:]
        eps_c = pp.tile([P, 1], F32, tag="epsc")
        nc.vector.memset(eps_c[:], EPS)
        nc.const_aps.aps[(F32, EPS)] = eps_c[:]
        ones_sb = pp.tile([P, 1], BF16, tag="ones")
        nc.vector.memset(ones_sb[:], 1.0)
        ident = pp.tile([P, P], BF16, tag="ident")
        make_identity(nc, ident[:])

        x3 = None

        with ExitStack() as lctx:
            lp = lctx.enter_context(tc.tile_pool(name="lpersist", bufs=1))
            wp = lctx.enter_context(tc.tile_pool(name="wpool", bufs=1))
            htp = lctx.enter_context(tc.tile_pool(name="htp", bufs=2))
            gp = lctx.enter_context(tc.tile_pool(name="gpool", bufs=5))
            g2p = lctx.enter_context(tc.tile_pool(name="g2pool", bufs=4))
            sgp = lctx.enter_context(tc.tile_pool(name="sgen", bufs=4))
            hp = lctx.enter_context(tc.tile_pool(name="hpool", bufs=1))
            wk = lctx.enter_context(tc.tile_pool(name="work", bufs=2))
            sm4 = lctx.enter_context(tc.tile_pool(name="v4", bufs=4))
            sm2 = lctx.enter_context(tc.tile_pool(name="v2", bufs=1))
            smr = lctx.enter_context(tc.tile_pool(name="strep", bufs=2))
            smo = lctx.enter_context(tc.tile_pool(name="stonce", bufs=1))
            spp = lctx.enter_context(tc.tile_pool(name="spart", bufs=2))

            dstc_sb = lp.tile([P, TT], BF16, tag="dstc")
            nc.sync.dma_start(dstc_sb[:], dt["dstcol"][:])
            iota_sb = lp.tile([P, P], BF16, tag="iota")
            nc.sync.dma_start(iota_sb[:], dt["iota_rep"][:])
            eidx_sb = lp.tile([P, TT * 8], I16, tag="eidx")
            nc.sync.dma_start(eidx_sb[:], dt["eidx"][:])

            # layer-0 x^T tiles: host-pretransposed
            xT = htp.tile([P, KC, NPC], BF16, tag="hT")
            nc.sync.dma_start(xT[:], dt["xT0"][:])

            s_part = t_part = tp_rep = None   # BN params of previous layer

            for l in range(L):
                gsrc = dt["x_bf"][:] if l == 0 else hrep_d[l - 1][:]

                w_sb = wp.tile([P, 2 * KC, HID], BF16, tag="wcat")
                nc.sync.dma_start(
                    w_sb[:], dt["wcat"][l].rearrange("p (k f) -> p k f",
                                                     k=2 * KC))

                h_sb = hp.tile([P, MC, HID], BF16, tag="h")
                if l < L - 1:
                    hTn = htp.tile([P, KC, NPC], BF16, tag="hT")
                else:
                    hTn = None

                st_sum = psS.tile([1, HID], F32, space="PSUM", tag="stsum")
                st_sq = psS.tile([1, HID], F32, space="PSUM", tag="stsq")

                for m in range(NB):
                    T_b = TB[m]
                    agg_ps = psA.tile([P, HID], F32, space="PSUM", tag="agg")
                    t0 = 0
                    tglob = 0
                    for piece, tcnt in enumerate(_pieces(T_b)):
                        if tcnt == 0:
                            continue
                        if l == 0:
                            psrc = gsrc
                        else:
                            # slice the source rows to the AG chunks this
                            # piece actually references, so early pieces
                            # need not wait for late AllGather chunks
                            mx = max(tile_maxchunk[m][t0:t0 + tcnt])
                            psrc = hrep_d[l - 1][:NC * 512 * (mx + 1), :]
                        G = gp.tile([P, PMAX, HID], BF16, tag="G")
                        nc.gpsimd.dma_gather(
                            G[:, :tcnt, :], psrc,
                            eidx_sb[:, (OFF[m] + t0) * 8:(OFF[m] + t0 + tcnt) * 8],
                            tcnt * P, tcnt * P, HID, single_packet=False,
                            queue_num=piece)
                        for t in range(tcnt):
                            if l == 0:
                                rhs = G[:, t, :]
                            else:
                                G2 = g2p.tile([P, HID], BF16, tag="G2")
                                nc.vector.tensor_tensor(
                                    G2[:], G[:, t, :], tp_rep[:], ADD)
                                nc.vector.tensor_scalar_max(G2[:], G2[:], 0.0)
                                rhs = G2[:]
                            S_t = sgp.tile([P, P], BF16, tag="St")
                            col = OFF[m] + t0 + t
                            nc.vector.tensor_tensor(
                                S_t[:], iota_sb[:],
                                dstc_sb[:, col:col + 1].to_broadcast((P, P)),
                                mybir.AluOpType.is_equal)
                            nc.tensor.matmul(
                                agg_ps[:], S_t[:],
                                rhs, start=(tglob == 0),
                                stop=(tglob == T_b - 1))
                            tglob += 1
                        t0 += tcnt
                    agg_sb = wk.tile([P, HID], BF16, tag="aggsb")
                    nc.scalar.activation(agg_sb[:], agg_ps[:], Copy)

                    # transpose agg (PE, via identity); fold in BN scale s
                    aggT = wk.tile([P, KC, P], BF16, tag="aggT")
                    for k in range(KC):
                        tr_ps = psT.tile([P, P], BF16, space="PSUM", tag="tr")
                        nc.tensor.transpose(tr_ps[:],
                                            agg_sb[:, k * P:(k + 1) * P],
                                            ident[:])
                        if l == 0:
                            nc.scalar.activation(aggT[:, k, :], tr_ps[:], Copy)
                        else:
                            nc.scalar.activation(aggT[:, k, :], tr_ps[:], Copy,
                                                 scale=s_part[:, k:k + 1])

                    # conv: h[m] = [x|agg] @ wcat
                    h_ps = psB.tile([P, HID], F32, space="PSUM", tag="conv")
                    for k in range(KC):
                        nc.tensor.matmul(h_ps[:], xT[:, k, m * P:(m + 1) * P],
                                         w_sb[:, k, :], start=(k == 0),
                                         stop=False)
                    for k in range(KC):
                        nc.tensor.matmul(h_ps[:], aggT[:, k, :],
                                         w_sb[:, KC + k, :], start=False,
                                         stop=(k == KC - 1))
                    nc.vector.tensor_copy(h_sb[:, m, :], h_ps[:])

                    hsq = wk.tile([P, HID], BF16, tag="hsq")
                    nc.scalar.activation(hsq[:], h_ps[:], Square)
                    nc.tensor.matmul(st_sum[:], ones_sb[:], h_sb[:, m, :],
                                     start=(m == 0), stop=(m == NB - 1))
                    nc.tensor.matmul(st_sq[:], ones_sb[:], hsq[:],
                                     start=(m == 0), stop=(m == NB - 1))

                    # h^T tiles for next layer's self term (PE transpose)
                    if hTn is not None:
                        for k in range(KC):
                            tr2 = psT.tile([P, P], BF16, space="PSUM", tag="tr")
                            nc.tensor.transpose(
                                tr2[:], h_sb[:, m, k * P:(k + 1) * P], ident[:])
                            nc.scalar.activation(
                                hTn[:, k, m * P:(m + 1) * P], tr2[:], Copy)

                    # chunked AllGather of pre-BN h (overlaps block compute);
                    # the last chunk goes AFTER the stats AllReduce dispatch
                    # so the AR isn't queued behind it on the CC engine
                    if l < L - 1 and m % 4 == 3 and m < NB - 1:
                        j = m // 4
                        nc.sync.dma_start(
                            ag_in_d[l][512 * j:512 * (j + 1), :].rearrange(
                                "(m p) f -> p m f", p=P),
                            h_sb[:, m - 3:m + 1, :])
                        nc.gpsimd.collective_compute(
                            "AllGather", mybir.AluOpType.bypass,
                            replica_groups=groups,
                            ins=[ag_in_d[l][512 * j:512 * (j + 1), :].opt()],
                            outs=[hrep_d[l][NC * 512 * j:
                                            NC * 512 * (j + 1), :].opt()])

                # ---- BN stats allreduce ----
                stat_sb = sm2.tile([1, 2 * HID], F32, tag="v2h")
                nc.vector.tensor_copy(stat_sb[:, :HID], st_sum[:])
                nc.vector.tensor_copy(stat_sb[:, HID:], st_sq[:])
                nc.sync.dma_start(st_in_d[l][:], stat_sb[:])
                nc.gpsimd.collective_compute(
                    "AllReduce", ADD, replica_groups=groups,
                    ins=[st_in_d[l][:].opt()], outs=[st_out_d[l][:].opt()])
                stat_r = sm2.tile([1, 2 * HID], F32, tag="v2h")
                nc.sync.dma_start(stat_r[:], st_out_d[l][:])

                # last AllGather chunk (blocks 12-15), after the AR dispatch
                if l < L - 1:
                    j = 3
                    nc.sync.dma_start(
                        ag_in_d[l][512 * j:512 * (j + 1), :].rearrange(
                            "(m p) f -> p m f", p=P),
                        h_sb[:, NB - 4:NB, :])
                    nc.gpsimd.collective_compute(
                        "AllGather", mybir.AluOpType.bypass,
                        replica_groups=groups,
                        ins=[ag_in_d[l][512 * j:512 * (j + 1), :].opt()],
                        outs=[hrep_d[l][NC * 512 * j:
                                        NC * 512 * (j + 1), :].opt()])

                # s = gamma / sqrt(var+eps); t = beta - mean*s; tp = t/s
                mean = sm4.tile([1, HID], F32, tag="v1h")
                nc.vector.tensor_scalar_mul(mean[:], stat_r[:, :HID], 1.0 / N)
                m2t = sm4.tile([1, HID], F32, tag="v1h")
                nc.vector.tensor_tensor(m2t[:], mean[:], mean[:], MULT)
                var = sm4.tile([1, HID], F32, tag="v1h")
                nc.vector.tensor_scalar(var[:], stat_r[:, HID:], 1.0 / N, None,
                                        MULT)
                nc.vector.tensor_tensor(var[:], var[:], m2t[:], SUB)
                std = sm4.tile([1, HID], F32, tag="v1h")
                nc.scalar.activation(std[:], var[:], Sqrt, bias=EPS)
                inv = sm4.tile([1, HID], F32, tag="v1h")
                nc.vector.reciprocal(inv[:], std[:])
                gam = sm4.tile([1, HID], F32, tag="v1h")
                nc.sync.dma_start(gam[:], dt["bn_g"][l][None, :])
                st_pack = sm2.tile([1, 3 * HID], F32, tag="v3h")
                nc.vector.tensor_tensor(st_pack[:, :HID], gam[:], inv[:], MULT)
                ms = sm4.tile([1, HID], F32, tag="v1h")
                nc.vector.tensor_tensor(ms[:], mean[:], st_pack[:, :HID], MULT)
                bet = sm4.tile([1, HID], F32, tag="v1h")
                nc.sync.dma_start(bet[:], dt["bn_b"][l][None, :])
                nc.vector.tensor_tensor(st_pack[:, HID:2 * HID], bet[:], ms[:],
                                        SUB)
                # tp = t / s  (= beta/s - mean; s > 0 since gamma > 0)
                sinv = sm4.tile([1, HID], F32, tag="v1h")
                nc.vector.reciprocal(sinv[:], st_pack[:, :HID])
                nc.vector.tensor_tensor(st_pack[:, 2 * HID:],
                                        st_pack[:, HID:2 * HID], sinv[:], MULT)
                st_dram = nc.dram_tensor(f"st_pack{l}", [1, 3 * HID], F32)
                nc.sync.dma_start(st_dram[:], st_pack[:])
                # bf16 copy of tp for the all-bf16 DVE fast path
                tp_bf = sm4.tile([1, HID], BF16, tag="tpbf")
                nc.vector.tensor_copy(tp_bf[:], st_pack[:, 2 * HID:])
                tp_dram = nc.dram_tensor(f"tp_bf{l}", [1, HID], BF16)
                nc.sync.dma_start(tp_dram[:], tp_bf[:])

                # per-partition [128, KC] views of s and t
                s_part = spp.tile([P, KC], F32, tag="sp")
                nc.sync.dma_start(
                    s_part[:],
                    st_dram[0:1, :HID].rearrange("o (k p) -> (o p) k", p=P))
                t_part = spp.tile([P, KC], F32, tag="tp")
                nc.sync.dma_start(
                    t_part[:],
                    st_dram[0:1, HID:2 * HID].rearrange("o (k p) -> (o p) k",
                                                        p=P))
                # replicated t/s row for consumer-side G bias
                tp_rep = smr.tile([P, HID], BF16, tag="tprep")
                nc.sync.dma_start(
                    tp_rep[:],
                    tp_dram[0:1, :].to_broadcast((P, HID)))

                if l < L - 1:
                    # turn h^T into x^T for next layer: relu(s*h + t),
                    # in place, per-partition scale/bias
                    for k in range(KC):
                        nc.scalar.activation(hTn[:, k, :], hTn[:, k, :], Relu,
                                             scale=s_part[:, k:k + 1],
                                             bias=t_part[:, k:k + 1])
                    xT = hTn
                else:
                    # final layer: normalize local x3 = relu(s*h + t)
                    s_rep = smo.tile([P, HID], F32, tag="srep")
                    nc.sync.dma_start(
                        s_rep[:], st_dram[0:1, :HID].to_broadcast((P, HID)))
                    t_rep = smo.tile([P, HID], F32, tag="trep")
                    nc.sync.dma_start(
                        t_rep[:],
                        st_dram[0:1, HID:2 * HID].to_broadcast((P, HID)))
                    x3 = xp.tile([P, MC, HID], BF16, tag="xnode")
                    for m in range(NB):
                        tmp = wk.tile([P, HID], F32, tag="norm")
                        nc.vector.tensor_tensor(tmp[:], h_sb[:, m, :],
                                                s_rep[:], MULT)
                        nc.vector.tensor_tensor(tmp[:], tmp[:], t_rep[:], ADD)
                        nc.scalar.activation(x3[:, m, :], tmp[:], Relu)

            # ---- AllToAll: redistribute x3 by node position ----
            x3v = x3[:].rearrange("p (g two) f -> p two g f", two=2)
            for d in range(NC):
                nc.sync.dma_start(
                    a2a_in_d[d * 256:(d + 1) * 256, :].rearrange(
                        "(pp g) f -> pp g f", g=8),
                    x3v[32 * (d % 4):32 * (d % 4) + 32, d // 4, :, :])
            nc.gpsimd.collective_compute(
                "AllToAll", mybir.AluOpType.bypass, replica_groups=groups,
                ins=[a2a_in_d[:].opt()], outs=[a2a_out_d[:].opt()])

        with ExitStack() as hctx:
            sm = hctx.enter_context(tc.tile_pool(name="once", bufs=1))
            wk2 = hctx.enter_context(tc.tile_pool(name="hwork", bufs=3))

            # ------------- graph path (local graphs) -------------
            pind = sm.tile([P, NB, 8], BF16, tag="pind")
            nc.sync.dma_start(
                pind[:],
                dt["pool_ind"][:].rearrange("p (m g) -> p m g", g=8))
            xgT = sm.tile([P, KC, 8], BF16, tag="xgT")
            for k in range(KC):
                pool_ps = psA.tile([P, HID], F32, space="PSUM", tag="agg")
                for m in range(NB):
                    nc.tensor.matmul(pool_ps[:, :8],
                                     x3[:, m, k * P:(k + 1) * P],
                                     pind[:, m, :], start=(m == 0),
                                     stop=(m == NB - 1))
                nc.scalar.activation(xgT[:, k, :], pool_ps[:, :8], Relu)

            def mlp_layer(src, w_dram, b_dram, kin, kout, act, tag):
                w_sb2 = sm.tile([P, kin * (kout * P)], BF16, tag=tag + "w")
                nc.sync.dma_start(w_sb2[:], w_dram[:])
                w3v = w_sb2[:].rearrange("p (k f) -> p k f", k=kin)
                b_sb = sm.tile([P, kout], F32, tag=tag + "b")
                nc.sync.dma_start(b_sb[:], b_dram[:])
                res = sm.tile([P, kout, 8], BF16, tag=tag + "o")
                for mo in range(kout):
                    ps = psB.tile([P, HID], F32, space="PSUM", tag="conv")
                    for k in range(kin):
                        nc.tensor.matmul(ps[:, :8],
                                         w3v[:, k, mo * P:(mo + 1) * P],
                                         src[:, k, :], start=(k == 0),
                                         stop=(k == kin - 1))
                    nc.scalar.activation(res[:, mo, :], ps[:, :8], act,
                                         bias=b_sb[:, mo:mo + 1])
                return res

            z1 = mlp_layer(xgT, dt["gs_w1"], dt["gs_b1"], KC, KC, Ident,
                           "gsw1")
            z2 = mlp_layer(z1, dt["gs_w2"], dt["gs_b2"], KC, KC, Relu,
                           "gsw2")
            g1 = mlp_layer(z2, dt["gh_w1"], dt["gh_b1"], KC, KC, Relu,
                           "ghw1")
            g2 = mlp_layer(g1, dt["gh_w2"], dt["gh_b2"], KC, GH2 // P,
                           Relu, "ghw2")
            w3_sb = sm.tile([P, 2 * GOUT], BF16, tag="ghw3")
            nc.sync.dma_start(w3_sb[:], dt["gh_w3"][:])
            b3_sb = sm.tile([GOUT, 1], F32, tag="ghb3")
            nc.sync.dma_start(b3_sb[:], dt["gh_b3"][:])
            g_ps = psB.tile([P, HID], F32, space="PSUM", tag="conv")
            for k in range(2):
                nc.tensor.matmul(g_ps[:GOUT, :8],
                                 w3_sb[:, k * GOUT:(k + 1) * GOUT],
                                 g2[:, k, :], start=(k == 0),
                                 stop=(k == 1))
            gsb = sm.tile([GOUT, B], F32, tag="gsb")
            nc.vector.memset(gsb[:], 0.0)
            nc.scalar.activation(gsb[:, :8], g_ps[:GOUT, :8], Ident,
                                 bias=b3_sb[:])
            nc.sync.dma_start(out_graph[:], gsb[:])

            # ------------- node head -------------
            nhw3_sb = sm.tile([P, POS_PC], BF16, tag="nhw3")
            nc.sync.dma_start(nhw3_sb[:], dt["nh_w3"][:])
            nhb1_sb = sm.tile([P, POS_PC * 2], F32, tag="nhb1")
            nc.sync.dma_start(nhb1_sb[:], dt["nh_b1"][:])
            nhb2_sb = sm.tile([P, POS_PC], F32, tag="nhb2")
            nc.sync.dma_start(nhb2_sb[:], dt["nh_b2"][:])
            nhb3_sb = sm.tile([POS_PC, 1], F32, tag="nhb3")
            nc.sync.dma_start(nhb3_sb[:], dt["nh_b3"][:])
            nhidx_sb = sm.tile([P, 16 * 8], I16, tag="nhidx")
            nc.sync.dma_start(nhidx_sb[:], dt["nh_idx"][:])

            nodeflat = sm.tile([1, POS_PC * B], F32, tag="nodeflat")
            for jj in range(16):
                xpT = wk2.tile([P, KC, P], BF16, tag="xpT")
                nc.gpsimd.dma_gather(xpT[:], a2a_out_d[:],
                                     nhidx_sb[:, jj * 8:(jj + 1) * 8],
                                     P, P, HID, transpose=True,
                                     queue_num=jj % 4)
                for half in range(2):
                    j = 2 * jj + half
                    w1_sb = wk2.tile([P, KC, NH1], BF16, tag="nhw1")
                    nc.sync.dma_start(
                        w1_sb[:],
                        dt["nh_w1"][j].rearrange("p (k f) -> p k f", k=KC))
                    w2_sb = wk2.tile([P, 2, NH2], BF16, tag="nhw2")
                    nc.sync.dma_start(
                        w2_sb[:],
                        dt["nh_w2"][j].rearrange("p (k f) -> p k f", k=2))
                    rhs = xpT[:, :, half * B:(half + 1) * B]
                    h1T = wk2.tile([P, 2, B], BF16, tag="h1T")
                    for mo in range(2):
                        ps = psB.tile([P, HID], F32, space="PSUM",
                                      tag="conv")
                        for k in range(KC):
                            nc.tensor.matmul(
                                ps[:, :B],
                                w1_sb[:, k, mo * P:(mo + 1) * P],
                                rhs[:, k, :], start=(k == 0),
                                stop=(k == KC - 1))
                        nc.scalar.activation(
                            h1T[:, mo, :], ps[:, :B], Relu,
                            bias=nhb1_sb[:, 2 * j + mo:2 * j + mo + 1])
                    ps2 = psB.tile([P, HID], F32, space="PSUM", tag="conv")
                    for k in range(2):
                        nc.tensor.matmul(ps2[:, :B], w2_sb[:, k, :],
                                         h1T[:, k, :], start=(k == 0),
                                         stop=(k == 1))
                    h2T = wk2.tile([P, B], BF16, tag="h2T")
                    nc.scalar.activation(h2T[:], ps2[:, :B], Relu,
                                         bias=nhb2_sb[:, j:j + 1])
                    ps3 = psA.tile([P, HID], F32, space="PSUM", tag="agg")
                    nc.tensor.matmul(ps3[:1, :B], nhw3_sb[:, j:j + 1],
                                     h2T[:], start=True, stop=True)
                    nc.scalar.activation(
                        nodeflat[0:1, j * B:(j + 1) * B], ps3[:1, :B],
                        Copy)
            nflat_d = nc.dram_tensor("nflat_d", [1, POS_PC * B], F32)
            nc.sync.dma_start(nflat_d[:], nodeflat[:])
            nodeT = sm.tile([POS_PC, B], F32, tag="nodeT")
            nc.sync.dma_start(
                nodeT[:],
                nflat_d[:].rearrange("o (j g) -> (o j) g", j=POS_PC))
            nodeS = sm.tile([POS_PC, B], F32, tag="nodeS")
            nc.vector.tensor_tensor(
                nodeS[:], nodeT[:],
                nhb3_sb[:].to_broadcast((POS_PC, B)), ADD)
            nc.sync.dma_start(out_node[:], nodeS[:])

    nc.compile()
    return nc


_PROG_CACHE = {}


def _get_program(TB):
    key = tuple(TB)
    if key not in _PROG_CACHE:
        _PROG_CACHE[key] = _build_program(TB)
    return _PROG_CACHE[key]


def kernel(**inputs):
    res = _run(inputs)
    return _assemble(res)


_LAST_RES = None


def _run(inputs, debug=False, trace=False):
    global _LAST_RES
    TB, maps = _host_prep(inputs)
    nc = _get_program(TB)
    res = run_bass_kernel_spmd(nc, maps, list(range(NC)), trace=trace)
    _LAST_RES = res
    if trace:
        print(f"HW exec time: {res.exec_time_ns} ns")
        print(f"mean exec time: {res.mean_exec_time_ns} ns "
              f"(max core {res.max_exec_time_core_id})")
    return res.results


def _assemble(results):
    full = np.empty((B, GOUT + NUM_NODES), np.float32)
    for c in range(NC):
        full[:, GOUT + POS_PC * c: GOUT + POS_PC * (c + 1)] = \
            results[c]["out_node"].T
        full[8 * c:8 * (c + 1), :GOUT] = results[c]["out_graph"][:, :8].T
    return full


# revision 44
# speedup vs baseline: 1.5248x; 1.2797x over previous
"""Trainium2 Bass kernel for nn_Base_55954833932808 (GNN message passing).

Distribution (8 NeuronCores):
  - Data-parallel over graphs: core c owns node rows [2048c, 2048c+2048)
    (graphs [8c, 8c+8)); conv weights replicated.
  - Node-head weights sharded along the node-position axis: core c owns
    positions [32c, 32c+32) for all 64 graphs (expert-parallel).
  - segment_sum(x[src], dst) per dst-shard: host sorts edges by dst, pads
    each 128-node block to T*128 edges; the device gathers x[src] rows
    (dma_gather, bf16, 4 SWDGE queues) and reduces on the TensorEngine with
    per-tile 0/1 selection matrices accumulated in PSUM.
  - Collective overlap: the pre-BN activations h are AllGathered in 4
    chunks as blocks finish (overlapping block compute); BatchNorm+ReLU is
    applied consumer-side next layer using relu(s*h+t) = s*relu(h + t/s)
    (valid: gamma > 0 for this problem), with s folded into the existing
    aggT PSUM->SBUF copy and into the x^T activation pass.
  - x^T tiles for the self-term come from PE transposes of local h (no
    gather); layer 0 uses a host-pretransposed x^T input.
  - The node head's input redistribution is an AllToAll (2 MB/core)
    instead of a full 16 MB AllGather.
"""

import numpy as np
import ml_dtypes

import concourse.bass as bass
import concourse.mybir as mybir
import concourse.tile as tile
from concourse import bacc
from concourse.masks import make_identity
from concourse.bass_utils import run_bass_kernel_spmd

# ---------------- problem constants (hardcoded per contest rules) -------------
NUM_NODES = 256
B = 64                  # graphs
N = NUM_NODES * B       # 16384 nodes
E = N * 16              # 262144 edges
HID = 512
L = 3
DSH = 512
GH1, GH2 = 512, 256
GOUT = 8
NH1, NH2 = 256, 128
EPS = 1e-5

NC = 8                  # cores
NPC = N // NC           # 2048 nodes per core
NB = 16                 # 128-node blocks per core
P = 128
KC = HID // P           # 4 feature chunks
MC = NPC // P           # 16 node chunks per core
POS_PC = NUM_NODES // NC  # 32 node-head positions per core

BF16 = mybir.dt.bfloat16
F32 = mybir.dt.float32
I16 = mybir.dt.int16

_nbf = ml_dtypes.bfloat16


def _wrap_idx(idx, width=None):
    """Wrap an index list into the dma_gather [128, n/16] layout
    (first 16 partitions hold the indices; replicated to all 8 Q7 groups)."""
    n = len(idx)
    assert n % 16 == 0
    w = n // 16
    if width is None:
        width = w
    out = np.zeros((128, width), np.int16)
    blk = np.asarray(idx, np.int16).reshape(w, 16).T  # [16, w]
    for r in range(8):
        out[16 * r:16 * r + 16, :w] = blk
    return out


def _pieces(t):
    """Split t tiles into 4 pieces for the 4 SWDGE queues."""
    q = t // 4
    return [t - 3 * q, q, q, q]


def _hrep_row(n):
    """Node id -> row in the chunk-major replicated-h layout.

    AllGather chunk j (blocks 4j..4j+3 of every core) must land contiguously,
    so hrep rows are ordered [chunk j][core c][block m%4][p]."""
    n = np.asarray(n)
    c = n // NPC
    m = (n % NPC) // P
    p = n % P
    return (m // 4) * (NC * 512) + c * 512 + (m % 4) * P + p


# ------------------------------ host preprocessing ---------------------------

def _host_prep(inputs):
    x = np.asarray(inputs["x"], np.float32)
    ei = np.asarray(inputs["edge_index"], np.int64)
    src, dst = ei[0], ei[1]

    # x rows permuted into the chunk-major hrep layout so one eidx table
    # works for layer 0 (reads x_bf) and layers 1-2 (read hrep)
    x_bf = np.empty((N, HID), _nbf)
    x_bf[_hrep_row(np.arange(N))] = x.astype(_nbf)

    # --- per-core edge structures (node-order ids, no permutation) ---
    order = np.argsort(dst, kind="stable")
    src_s, dst_s = src[order], dst[order]
    core_of = dst_s // NPC

    per_core = []
    counts = np.zeros((NC, NB), np.int64)
    for c in range(NC):
        sel = core_of == c
        s_c, d_c = src_s[sel], dst_s[sel] - c * NPC
        blocks = []
        for b in range(NB):
            bs = d_c // P == b
            blocks.append((s_c[bs], d_c[bs] - b * P))
            counts[c, b] = bs.sum()
        per_core.append(blocks)

    TB = tuple(int(np.ceil(counts[:, b].max() / P)) for b in range(NB))
    OFF = np.concatenate([[0], np.cumsum(TB)])  # tile offsets per block
    TT = int(OFF[-1])

    # dstcol[p, tile] = within-block dst of edge p of that tile (-1 = pad);
    # the device builds the one-hot S tile as (iota == dstcol) on the fly.
    # Edges within a block are sorted by source AG chunk so that gather
    # pieces referencing only early chunks need not wait for the last
    # AllGather chunk of the previous layer.
    dst_all, idx_all, deg_all = [], [], []
    # tile_maxchunk[b][t] = max source chunk referenced by tile t of block b
    tile_maxchunk = [[0] * TB[b] for b in range(NB)]
    for c in range(NC):
        dstcol = np.full((128, TT), -1.0, np.float32)
        idxw = np.zeros((128, TT * 8), np.int16)
        deg = np.zeros((1, NB * P), np.float32)
        for b in range(NB):
            s_b, n_b = per_core[c][b]
            rows = _hrep_row(s_b)
            chunk = rows // (NC * 512)
            o2 = np.argsort(chunk, kind="stable")
            rows, n_b, chunk = rows[o2], n_b[o2], chunk[o2]
            cnt = len(rows)
            TW = TB[b] * P
            pad_idx = np.zeros(TW, np.int64)
            pad_idx[:cnt] = rows            # pads -> row 0 (chunk 0)
            idxw[:, OFF[b] * 8:OFF[b + 1] * 8] = _wrap_idx(pad_idx, TW // 16)
            tt = np.arange(cnt) // P
            pp = np.arange(cnt) % P
            dstcol[pp, int(OFF[b]) + tt] = n_b
            deg[0, b * P:(b + 1) * P] = np.bincount(n_b, minlength=P)
            for t in range(TB[b]):
                ch = chunk[t * P:(t + 1) * P]
                mx = int(ch.max()) if len(ch) else 0
                tile_maxchunk[b][t] = max(tile_maxchunk[b][t], mx)
        dst_all.append(dstcol)
        idx_all.append(idxw)
        deg_all.append(deg.astype(_nbf))

    iota_rep = np.broadcast_to(np.arange(P, dtype=np.float32),
                               (P, P)).astype(_nbf).copy()

    # --- weights ---
    def chunked(w):  # [Kin, F] -> [128, (Kin/128)*F]
        ki, f = w.shape
        return np.ascontiguousarray(
            w.reshape(ki // P, P, f).transpose(1, 0, 2).reshape(P, -1)).astype(_nbf)

    wcat = np.zeros((L, P, 2 * KC * HID), _nbf)
    for l in range(L):
        wself = np.asarray(inputs["conv_wself"][l], np.float32)
        wnei = np.asarray(inputs["conv_wneigh"][l], np.float32)
        wcat[l] = chunked(np.concatenate([wself, wnei], axis=0))

    gs_w1 = chunked(np.asarray(inputs["gs_w1"], np.float32) / NUM_NODES)
    gs_w2 = chunked(np.asarray(inputs["gs_w2"], np.float32))
    gh_w1 = chunked(np.asarray(inputs["gh_w1"], np.float32))
    gh_w2 = chunked(np.asarray(inputs["gh_w2"], np.float32))
    gh_w3 = chunked(np.asarray(inputs["gh_w3"], np.float32))

    def pcol(b):  # [F] -> [128, F/128] f32 (per-partition bias columns)
        return np.ascontiguousarray(
            np.asarray(b, np.float32).reshape(-1, P).T)

    gs_b1 = pcol(inputs["gs_b1"]); gs_b2 = pcol(inputs["gs_b2"])
    gh_b1 = pcol(inputs["gh_b1"]); gh_b2 = pcol(inputs["gh_b2"])
    gh_b3 = np.asarray(inputs["gh_b3"], np.float32).reshape(GOUT, 1)

    pool_ind = np.zeros((P, NB * 8), _nbf)
    for m in range(NB):
        pool_ind[:, m * 8 + m // 2] = 1.0

    bn_g = np.asarray(inputs["bn_gamma"], np.float32)
    bn_b = np.asarray(inputs["bn_beta"], np.float32)

    # --- node-head (per core) ---
    nh_w1 = np.asarray(inputs["nh_w1"], np.float32)   # [256, 512, 256]
    nh_w2 = np.asarray(inputs["nh_w2"], np.float32)   # [256, 256, 128]
    nh_w3 = np.asarray(inputs["nh_w3"], np.float32)   # [256, 128, 1]
    nh_b1 = np.asarray(inputs["nh_b1"], np.float32)
    nh_b2 = np.asarray(inputs["nh_b2"], np.float32)
    nh_b3 = np.asarray(inputs["nh_b3"], np.float32)

    # node-head gather from the AllToAll output buffer:
    # a2a_out row s*256 + pp*8 + g = x3 of (graph 8s+g, position 32c+pp)
    nhidx = np.zeros((128, 16 * 8), np.int16)
    for jj in range(16):
        ids = []
        for half in range(2):
            pp = 2 * jj + half
            for s in range(NC):
                for g in range(8):
                    ids.append(s * 256 + pp * 8 + g)
        nhidx[:, jj * 8:(jj + 1) * 8] = _wrap_idx(ids, 8)

    per_core_maps = []
    for c in range(NC):
        pos = np.arange(POS_PC) + POS_PC * c
        w1 = np.stack([chunked(nh_w1[p]) for p in pos])          # [32,128,4*256]
        w2 = np.stack([chunked(nh_w2[p]) for p in pos])          # [32,128,2*128]
        w3 = np.ascontiguousarray(nh_w3[pos, :, 0].T).astype(_nbf)  # [128, 32]
        b1 = np.zeros((P, POS_PC * 2), np.float32)
        for j in range(POS_PC):
            b1[:, 2 * j] = nh_b1[pos[j], :P]
            b1[:, 2 * j + 1] = nh_b1[pos[j], P:]
        b2 = np.ascontiguousarray(nh_b2[pos].T)                  # [128, 32]
        b3 = nh_b3[pos].reshape(POS_PC, 1)

        # own-shard x^T tiles: xT0[p, k, i] = x[c*NPC + i][k*128 + p]
        xs = x[c * NPC:(c + 1) * NPC]                            # [2048, 512]
        xT0 = np.ascontiguousarray(
            xs.reshape(NPC, KC, P).transpose(2, 1, 0)).astype(_nbf)

        per_core_maps.append(dict(
            x_bf=x_bf, xT0=xT0,
            dstcol=dst_all[c].astype(_nbf), eidx=idx_all[c],
            iota_rep=iota_rep, deg=deg_all[c],
            wcat=wcat, bn_g=bn_g, bn_b=bn_b,
            gs_w1=gs_w1, gs_w2=gs_w2, gh_w1=gh_w1, gh_w2=gh_w2, gh_w3=gh_w3,
            gs_b1=gs_b1, gs_b2=gs_b2, gh_b1=gh_b1, gh_b2=gh_b2, gh_b3=gh_b3,
            pool_ind=pool_ind,
            nh_w1=w1, nh_w2=w2, nh_w3=w3, nh_b1=b1, nh_b2=b2, nh_b3=b3,
            nh_idx=nhidx,
        ))
    return (TB, tuple(tuple(t) for t in tile_maxchunk)), per_core_maps


# ------------------------------ device program --------------------------------

def _build_program(key):
    TB, tile_maxchunk = key
    TB = tuple(TB)
    OFF = [0]
    for t in TB:
        OFF.append(OFF[-1] + t)
    TT = OFF[-1]
    TMAX = max(TB)
    PMAX = max(_pieces(TMAX))

    nc = bacc.Bacc("TRN2", target_bir_lowering=False, debug=False,
                   num_devices=NC, num_swdge_queues=4)

    dt = {}
    def din(name, shape, dtype):
        dt[name] = nc.dram_tensor(name, list(shape), dtype, kind="ExternalInput")
        return dt[name]

    din("x_bf", (N, HID), BF16)
    din("xT0", (P, KC, NPC), BF16)
    din("dstcol", (P, TT), BF16)
    din("iota_rep", (P, P), BF16)
    din("deg", (1, NB * P), BF16)
    din("eidx", (P, TT * 8), I16)
    din("wcat", (L, P, 2 * KC * HID), BF16)
    din("bn_g", (L, HID), F32)
    din("bn_b", (L, HID), F32)
    din("gs_w1", (P, KC * DSH), BF16)
    din("gs_w2", (P, KC * DSH), BF16)
    din("gh_w1", (P, KC * GH1), BF16)
    din("gh_w2", (P, KC * GH2), BF16)
    din("gh_w3", (P, 2 * GOUT), BF16)
    din("gs_b1", (P, KC), F32)
    din("gs_b2", (P, KC), F32)
    din("gh_b1", (P, KC), F32)
    din("gh_b2", (P, GH2 // P), F32)
    din("gh_b3", (GOUT, 1), F32)
    din("pool_ind", (P, NB * 8), BF16)
    din("nh_w1", (POS_PC, P, KC * NH1), BF16)
    din("nh_w2", (POS_PC, P, 2 * NH2), BF16)
    din("nh_w3", (P, POS_PC), BF16)
    din("nh_b1", (P, POS_PC * 2), F32)
    din("nh_b2", (P, POS_PC), F32)
    din("nh_b3", (POS_PC, 1), F32)
    din("nh_idx", (P, 16 * 8), I16)

    out_node = nc.dram_tensor("out_node", [POS_PC, B], F32, kind="ExternalOutput")
    out_graph = nc.dram_tensor("out_graph", [GOUT, B], F32, kind="ExternalOutput")

    # collective buffers (internal DRAM, Shared address space)
    st_in_d, st_out_d = [], []
    ag_in_d, hrep_d = [], []
    for l in range(L):
        st_in_d.append(nc.dram_tensor(f"st_in{l}", [1, 2 * HID], F32))
        st_out_d.append(nc.dram_tensor(f"st_out{l}", [1, 2 * HID], F32,
                                       addr_space="Shared"))
    for l in range(L - 1):
        ag_in_d.append(nc.dram_tensor(f"ag_in{l}", [NPC, HID], BF16))
        hrep_d.append(nc.dram_tensor(f"hrep{l}", [N, HID], BF16,
                                     addr_space="Shared"))
    a2a_in_d = nc.dram_tensor("a2a_in", [NPC, HID], BF16)
    a2a_out_d = nc.dram_tensor("a2a_out", [NPC, HID], BF16)

    groups = [list(range(NC))]
    Relu = mybir.ActivationFunctionType.Relu
    Copy = mybir.ActivationFunctionType.Copy
    Ident = mybir.ActivationFunctionType.Identity
    Square = mybir.ActivationFunctionType.Square
    Sqrt = mybir.ActivationFunctionType.Sqrt
    ADD = mybir.AluOpType.add
    MULT = mybir.AluOpType.mult
    SUB = mybir.AluOpType.subtract

    from contextlib import ExitStack
    with tile.TileContext(nc) as tc, ExitStack() as octx:
        pp = octx.enter_context(tc.tile_pool(name="outer", bufs=1))
        xp = octx.enter_context(tc.tile_pool(name="xnode", bufs=1))
        psA = octx.enter_context(tc.tile_pool(name="psA", bufs=2, space="PSUM"))
        psB = octx.enter_context(tc.tile_pool(name="psB", bufs=2, space="PSUM"))
        psS = octx.enter_context(tc.tile_pool(name="psS", bufs=1, space="PSUM"))
        psT = octx.enter_context(tc.tile_pool(name="psT", bufs=2, space="PSUM"))

        # const APs for activation biases
        zero_c = pp.tile([P, 1], F32, tag="zeroc")
        nc.vector.memset(zero_c[:], 0.0)
        nc.const_aps.aps[(F32, 0.0)] = zero_c[:]
        eps_c = pp.tile([P, 1], F32, tag="epsc")
        nc.vector.memset(eps_c[:], EPS)
        nc.const_aps.aps[(F32, EPS)] = eps_c[:]
        ones_sb = pp.tile([P, 1], BF16, tag="ones")
        nc.vector.memset(ones_sb[:], 1.0)
        ident = pp.tile([P, P], BF16, tag="ident")
        make_identity(nc, ident[:])

        x3 = None

        with ExitStack() as lctx:
            lp = lctx.enter_context(tc.tile_pool(name="lpersist", bufs=1))
            wp = lctx.enter_context(tc.tile_pool(name="wpool", bufs=1))
            htp = lctx.enter_context(tc.tile_pool(name="htp", bufs=2))
            gp = lctx.enter_context(tc.tile_pool(name="gpool", bufs=5))
            g2p = lctx.enter_context(tc.tile_pool(name="g2pool", bufs=4))
            sgp = lctx.enter_context(tc.tile_pool(name="sgen", bufs=4))
            hp = lctx.enter_context(tc.tile_pool(name="hpool", bufs=1))
            wk = lctx.enter_context(tc.tile_pool(name="work", bufs=2))
            sm4 = lctx.enter_context(tc.tile_pool(name="v4", bufs=4))
            sm2 = lctx.enter_context(tc.tile_pool(name="v2", bufs=1))
            smr = lctx.enter_context(tc.tile_pool(name="strep", bufs=2))
            smo = lctx.enter_context(tc.tile_pool(name="stonce", bufs=1))
            spp = lctx.enter_context(tc.tile_pool(name="spart", bufs=2))

            dstc_sb = lp.tile([P, TT], BF16, tag="dstc")
            nc.sync.dma_start(dstc_sb[:], dt["dstcol"][:])
            iota_sb = lp.tile([P, P], BF16, tag="iota")
            nc.sync.dma_start(iota_sb[:], dt["iota_rep"][:])
            eidx_sb = lp.tile([P, TT * 8], I16, tag="eidx")
            nc.sync.dma_start(eidx_sb[:], dt["eidx"][:])
            deg_sb = lp.tile([1, NB * P], BF16, tag="deg")
            nc.sync.dma_start(deg_sb[:], dt["deg"][:])

            # layer-0 x^T tiles: host-pretransposed
            xT = htp.tile([P, KC, NPC], BF16, tag="hT")
            nc.sync.dma_start(xT[:], dt["xT0"][:])

            # BN params of the previous layer (consumer-side application)
            s_part = t_part = ntp_rep = tp_bf_prev = None

            for l in range(L):
                gsrc = dt["x_bf"][:] if l == 0 else hrep_d[l - 1][:]

                w_sb = wp.tile([P, 2 * KC, HID], BF16, tag="wcat")
                nc.sync.dma_start(
                    w_sb[:], dt["wcat"][l].rearrange("p (k f) -> p k f",
                                                     k=2 * KC))

                h_sb = hp.tile([P, MC, HID], BF16, tag="h")
                if l < L - 1:
                    hTn = htp.tile([P, KC, NPC], BF16, tag="hT")
                else:
                    hTn = None

                st_sum = psS.tile([1, HID], F32, space="PSUM", tag="stsum")
                st_sq = psS.tile([1, HID], F32, space="PSUM", tag="stsq")

                for m in range(NB):
                    T_b = TB[m]
                    agg_ps = psA.tile([P, HID], F32, space="PSUM", tag="agg")
                    t0 = 0
                    tglob = 0
                    for piece, tcnt in enumerate(_pieces(T_b)):
                        if tcnt == 0:
                            continue
                        if l == 0:
                            psrc = gsrc
                        else:
                            # slice the source rows to the AG chunks this
                            # piece actually references, so early pieces
                            # need not wait for late AllGather chunks
                            mx = max(tile_maxchunk[m][t0:t0 + tcnt])
                            psrc = hrep_d[l - 1][:NC * 512 * (mx + 1), :]
                        G = gp.tile([P, PMAX, HID], BF16, tag="G")
                        nc.gpsimd.dma_gather(
                            G[:, :tcnt, :], psrc,
                            eidx_sb[:, (OFF[m] + t0) * 8:(OFF[m] + t0 + tcnt) * 8],
                            tcnt * P, tcnt * P, HID, single_packet=False,
                            queue_num=piece)
                        if l > 0:
                            # relu(G + tp) = max(G, -tp) + tp; the +tp part
                            # becomes the rank-1 deg x tp matmul below
                            G2 = g2p.tile([P, PMAX, HID], BF16, tag="G2")
                            nc.vector.tensor_tensor(
                                G2[:, :tcnt, :], G[:, :tcnt, :],
                                ntp_rep[:].rearrange("p (o f) -> p o f", o=1)
                                .to_broadcast((P, tcnt, HID)),
                                mybir.AluOpType.max)
                            Gc = G2
                        else:
                            Gc = G
                        S_p = sgp.tile([P, PMAX, P], BF16, tag="St")
                        c0 = OFF[m] + t0
                        nc.vector.tensor_tensor(
                            S_p[:, :tcnt, :],
                            iota_sb[:].rearrange("p (o n) -> p o n", o=1)
                            .to_broadcast((P, tcnt, P)),
                            dstc_sb[:, c0:c0 + tcnt].rearrange(
                                "p (t o) -> p t o", o=1)
                            .to_broadcast((P, tcnt, P)),
                            mybir.AluOpType.is_equal)
                        for t in range(tcnt):
                            nc.tensor.matmul(
                                agg_ps[:], S_p[:, t, :],
                                Gc[:, t, :], start=(tglob == 0),
                                stop=(l == 0 and tglob == T_b - 1))
                            tglob += 1
                        t0 += tcnt
                    if l > 0:
                        # agg += deg (x) tp   (rank-1, restores the +tp term)
                        nc.tensor.matmul(
                            agg_ps[:], deg_sb[:, m * P:(m + 1) * P],
                            tp_bf_prev[:], start=False, stop=True)
                    agg_sb = wk.tile([P, HID], BF16, tag="aggsb")
                    nc.scalar.activation(agg_sb[:], agg_ps[:], Copy)

                    # transpose agg (PE, via identity); fold in BN scale s
                    aggT = wk.tile([P, KC, P], BF16, tag="aggT")
                    for k in range(KC):
                        tr_ps = psT.tile([P, P], BF16, space="PSUM", tag="tr")
                        nc.tensor.transpose(tr_ps[:],
                                            agg_sb[:, k * P:(k + 1) * P],
                                            ident[:])
                        if l == 0:
                            nc.scalar.activation(aggT[:, k, :], tr_ps[:], Copy)
                        else:
                            nc.scalar.activation(aggT[:, k, :], tr_ps[:], Copy,
                                                 scale=s_part[:, k:k + 1])

                    # conv: h[m] = [x|agg] @ wcat
                    h_ps = psB.tile([P, HID], F32, space="PSUM", tag="conv")
                    for k in range(KC):
                        nc.tensor.matmul(h_ps[:], xT[:, k, m * P:(m + 1) * P],
                                         w_sb[:, k, :], start=(k == 0),
                                         stop=False)
                    for k in range(KC):
                        nc.tensor.matmul(h_ps[:], aggT[:, k, :],
                                         w_sb[:, KC + k, :], start=False,
                                         stop=(k == KC - 1))
                    nc.scalar.activation(h_sb[:, m, :], h_ps[:], Copy)

                    hsq = wk.tile([P, HID], BF16, tag="hsq")
                    nc.scalar.activation(hsq[:], h_ps[:], Square)
                    nc.tensor.matmul(st_sum[:], ones_sb[:], h_sb[:, m, :],
                                     start=(m == 0), stop=(m == NB - 1))
                    nc.tensor.matmul(st_sq[:], ones_sb[:], hsq[:],
                                     start=(m == 0), stop=(m == NB - 1))

                    # h^T tiles for next layer's self term (PE transpose)
                    if hTn is not None:
                        for k in range(KC):
                            tr2 = psT.tile([P, P], BF16, space="PSUM", tag="tr")
                            nc.tensor.transpose(
                                tr2[:], h_sb[:, m, k * P:(k + 1) * P], ident[:])
                            nc.scalar.activation(
                                hTn[:, k, m * P:(m + 1) * P], tr2[:], Copy)

                    # chunked AllGather of pre-BN h (overlaps block compute);
                    # the last chunk goes AFTER the stats AllReduce dispatch
                    # so the AR isn't queued behind it on the CC engine
                    if l < L - 1 and m % 4 == 3 and m < NB - 1:
                        j = m // 4
                        nc.sync.dma_start(
                            ag_in_d[l][512 * j:512 * (j + 1), :].rearrange(
                                "(m p) f -> p m f", p=P),
                            h_sb[:, m - 3:m + 1, :])
                        nc.gpsimd.collective_compute(
                            "AllGather", mybir.AluOpType.bypass,
                            replica_groups=groups,
                            ins=[ag_in_d[l][512 * j:512 * (j + 1), :].opt()],
                            outs=[hrep_d[l][NC * 512 * j:
                                            NC * 512 * (j + 1), :].opt()])

                # ---- BN stats allreduce ----
                stat_sb = sm2.tile([1, 2 * HID], F32, tag="v2h")
                nc.scalar.activation(stat_sb[:, :HID], st_sum[:], Copy)
                nc.scalar.activation(stat_sb[:, HID:], st_sq[:], Copy)
                nc.sync.dma_start(st_in_d[l][:], stat_sb[:])
                nc.gpsimd.collective_compute(
                    "AllReduce", ADD, replica_groups=groups,
                    ins=[st_in_d[l][:].opt()], outs=[st_out_d[l][:].opt()])
                stat_r = sm2.tile([1, 2 * HID], F32, tag="v2h")
                nc.sync.dma_start(stat_r[:], st_out_d[l][:])

                # last AllGather chunk (blocks 12-15), after the AR dispatch
                if l < L - 1:
                    j = 3
                    nc.sync.dma_start(
                        ag_in_d[l][512 * j:512 * (j + 1), :].rearrange(
                            "(m p) f -> p m f", p=P),
                        h_sb[:, NB - 4:NB, :])
                    nc.gpsimd.collective_compute(
                        "AllGather", mybir.AluOpType.bypass,
                        replica_groups=groups,
                        ins=[ag_in_d[l][512 * j:512 * (j + 1), :].opt()],
                        outs=[hrep_d[l][NC * 512 * j:
                                        NC * 512 * (j + 1), :].opt()])

                # s = gamma / sqrt(var+eps); t = beta - mean*s; tp = t/s
                mean = sm4.tile([1, HID], F32, tag="v1h")
                nc.vector.tensor_scalar_mul(mean[:], stat_r[:, :HID], 1.0 / N)
                m2t = sm4.tile([1, HID], F32, tag="v1h")
                nc.vector.tensor_tensor(m2t[:], mean[:], mean[:], MULT)
                var = sm4.tile([1, HID], F32, tag="v1h")
                nc.vector.tensor_scalar(var[:], stat_r[:, HID:], 1.0 / N, None,
                                        MULT)
                nc.vector.tensor_tensor(var[:], var[:], m2t[:], SUB)
                std = sm4.tile([1, HID], F32, tag="v1h")
                nc.scalar.activation(std[:], var[:], Sqrt, bias=EPS)
                inv = sm4.tile([1, HID], F32, tag="v1h")
                nc.vector.reciprocal(inv[:], std[:])
                gam = sm4.tile([1, HID], F32, tag="v1h")
                nc.sync.dma_start(gam[:], dt["bn_g"][l][None, :])
                st_pack = sm2.tile([1, 3 * HID], F32, tag="v3h")
                nc.vector.tensor_tensor(st_pack[:, :HID], gam[:], inv[:], MULT)
                ms = sm4.tile([1, HID], F32, tag="v1h")
                nc.vector.tensor_tensor(ms[:], mean[:], st_pack[:, :HID], MULT)
                bet = sm4.tile([1, HID], F32, tag="v1h")
                nc.sync.dma_start(bet[:], dt["bn_b"][l][None, :])
                nc.vector.tensor_tensor(st_pack[:, HID:2 * HID], bet[:], ms[:],
                                        SUB)
                # tp = t / s  (= beta/s - mean; s > 0 since gamma > 0)
                sinv = sm4.tile([1, HID], F32, tag="v1h")
                nc.vector.reciprocal(sinv[:], st_pack[:, :HID])
                nc.vector.tensor_tensor(st_pack[:, 2 * HID:],
                                        st_pack[:, HID:2 * HID], sinv[:], MULT)
                st_dram = nc.dram_tensor(f"st_pack{l}", [1, 3 * HID], F32)
                nc.sync.dma_start(st_dram[:], st_pack[:])
                # bf16 tp row (for the rank-1 deg matmul) and -tp (for the
                # per-tile max); both bf16 for the DVE fast path
                tp_bf = sm4.tile([1, HID], BF16, tag="tpbf")
                nc.vector.tensor_copy(tp_bf[:], st_pack[:, 2 * HID:])
                ntp_bf = sm4.tile([1, HID], BF16, tag="ntpbf")
                nc.vector.tensor_scalar(ntp_bf[:], st_pack[:, 2 * HID:],
                                        -1.0, None, MULT)
                ntp_dram = nc.dram_tensor(f"ntp_bf{l}", [1, HID], BF16)
                nc.sync.dma_start(ntp_dram[:], ntp_bf[:])

                # per-partition [128, KC] views of s and t
                s_part = spp.tile([P, KC], F32, tag="sp")
                nc.sync.dma_start(
                    s_part[:],
                    st_dram[0:1, :HID].rearrange("o (k p) -> (o p) k", p=P))
                t_part = spp.tile([P, KC], F32, tag="tp")
                nc.sync.dma_start(
                    t_part[:],
                    st_dram[0:1, HID:2 * HID].rearrange("o (k p) -> (o p) k",
                                                        p=P))
                # replicated -t/s row for the consumer-side max
                ntp_rep = smr.tile([P, HID], BF16, tag="ntprep")
                nc.sync.dma_start(
                    ntp_rep[:],
                    ntp_dram[0:1, :].to_broadcast((P, HID)))
                tp_bf_prev = tp_bf

                if l < L - 1:
                    # turn h^T into x^T for next layer: relu(s*h + t),
                    # in place, per-partition scale/bias
                    for k in range(KC):
                        nc.scalar.activation(hTn[:, k, :], hTn[:, k, :], Relu,
                                             scale=s_part[:, k:k + 1],
                                             bias=t_part[:, k:k + 1])
                    xT = hTn
                else:
                    # final layer: normalize local x3 = relu(s*h + t)
                    s_rep = smo.tile([P, HID], F32, tag="srep")
                    nc.sync.dma_start(
                        s_rep[:], st_dram[0:1, :HID].to_broadcast((P, HID)))
                    t_rep = smo.tile([P, HID], F32, tag="trep")
                    nc.sync.dma_start(
                        t_rep[:],
                        st_dram[0:1, HID:2 * HID].to_broadcast((P, HID)))
                    x3 = xp.tile([P, MC, HID], BF16, tag="xnode")
                    for m in range(NB):
                        tmp = wk.tile([P, HID], F32, tag="norm")
                        nc.vector.tensor_tensor(tmp[:], h_sb[:, m, :],
                                                s_rep[:], MULT)
                        nc.vector.tensor_tensor(tmp[:], tmp[:], t_rep[:], ADD)
                        nc.scalar.activation(x3[:, m, :], tmp[:], Relu)

            # ---- AllToAll: redistribute x3 by node position ----
            x3v = x3[:].rearrange("p (g two) f -> p two g f", two=2)
            for d in range(NC):
                nc.sync.dma_start(
                    a2a_in_d[d * 256:(d + 1) * 256, :].rearrange(
                        "(pp g) f -> pp g f", g=8),
                    x3v[32 * (d % 4):32 * (d % 4) + 32, d // 4, :, :])
            nc.gpsimd.collective_compute(
                "AllToAll", mybir.AluOpType.bypass, replica_groups=groups,
                ins=[a2a_in_d[:].opt()], outs=[a2a_out_d[:].opt()])

        with ExitStack() as hctx:
            sm = hctx.enter_context(tc.tile_pool(name="once", bufs=1))
            wk2 = hctx.enter_context(tc.tile_pool(name="hwork", bufs=3))

            # ------------- graph path (local graphs) -------------
            pind = sm.tile([P, NB, 8], BF16, tag="pind")
            nc.sync.dma_start(
                pind[:],
                dt["pool_ind"][:].rearrange("p (m g) -> p m g", g=8))
            xgT = sm.tile([P, KC, 8], BF16, tag="xgT")
            for k in range(KC):
                pool_ps = psA.tile([P, HID], F32, space="PSUM", tag="agg")
                for m in range(NB):
                    nc.tensor.matmul(pool_ps[:, :8],
                                     x3[:, m, k * P:(k + 1) * P],
                                     pind[:, m, :], start=(m == 0),
                                     stop=(m == NB - 1))
                nc.scalar.activation(xgT[:, k, :], pool_ps[:, :8], Relu)

            def mlp_layer(src, w_dram, b_dram, kin, kout, act, tag):
                w_sb2 = sm.tile([P, kin * (kout * P)], BF16, tag=tag + "w")
                nc.sync.dma_start(w_sb2[:], w_dram[:])
                w3v = w_sb2[:].rearrange("p (k f) -> p k f", k=kin)
                b_sb = sm.tile([P, kout], F32, tag=tag + "b")
                nc.sync.dma_start(b_sb[:], b_dram[:])
                res = sm.tile([P, kout, 8], BF16, tag=tag + "o")
                for mo in range(kout):
                    ps = psB.tile([P, HID], F32, space="PSUM", tag="conv")
                    for k in range(kin):
                        nc.tensor.matmul(ps[:, :8],
                                         w3v[:, k, mo * P:(mo + 1) * P],
                                         src[:, k, :], start=(k == 0),
                                         stop=(k == kin - 1))
                    nc.scalar.activation(res[:, mo, :], ps[:, :8], act,
                                         bias=b_sb[:, mo:mo + 1])
                return res

            z1 = mlp_layer(xgT, dt["gs_w1"], dt["gs_b1"], KC, KC, Ident,
                           "gsw1")
            z2 = mlp_layer(z1, dt["gs_w2"], dt["gs_b2"], KC, KC, Relu,
                           "gsw2")
            g1 = mlp_layer(z2, dt["gh_w1"], dt["gh_b1"], KC, KC, Relu,
                           "ghw1")
            g2 = mlp_layer(g1, dt["gh_w2"], dt["gh_b2"], KC, GH2 // P,
                           Relu, "ghw2")
            w3_sb = sm.tile([P, 2 * GOUT], BF16, tag="ghw3")
            nc.sync.dma_start(w3_sb[:], dt["gh_w3"][:])
            b3_sb = sm.tile([GOUT, 1], F32, tag="ghb3")
            nc.sync.dma_start(b3_sb[:], dt["gh_b3"][:])
            g_ps = psB.tile([P, HID], F32, space="PSUM", tag="conv")
            for k in range(2):
                nc.tensor.matmul(g_ps[:GOUT, :8],
                                 w3_sb[:, k * GOUT:(k + 1) * GOUT],
                                 g2[:, k, :], start=(k == 0),
                                 stop=(k == 1))
            gsb = sm.tile([GOUT, B], F32, tag="gsb")
            nc.vector.memset(gsb[:], 0.0)
            nc.scalar.activation(gsb[:, :8], g_ps[:GOUT, :8], Ident,
                                 bias=b3_sb[:])
            nc.sync.dma_start(out_graph[:], gsb[:])

            # ------------- node head -------------
            nhw3_sb = sm.tile([P, POS_PC], BF16, tag="nhw3")
            nc.sync.dma_start(nhw3_sb[:], dt["nh_w3"][:])
            nhb1_sb = sm.tile([P, POS_PC * 2], F32, tag="nhb1")
            nc.sync.dma_start(nhb1_sb[:], dt["nh_b1"][:])
            nhb2_sb = sm.tile([P, POS_PC], F32, tag="nhb2")
            nc.sync.dma_start(nhb2_sb[:], dt["nh_b2"][:])
            nhb3_sb = sm.tile([POS_PC, 1], F32, tag="nhb3")
            nc.sync.dma_start(nhb3_sb[:], dt["nh_b3"][:])
            nhidx_sb = sm.tile([P, 16 * 8], I16, tag="nhidx")
            nc.sync.dma_start(nhidx_sb[:], dt["nh_idx"][:])

            nodeflat = sm.tile([1, POS_PC * B], F32, tag="nodeflat")
            for jj in range(16):
                xpT = wk2.tile([P, KC, P], BF16, tag="xpT")
                nc.gpsimd.dma_gather(xpT[:], a2a_out_d[:],
                                     nhidx_sb[:, jj * 8:(jj + 1) * 8],
                                     P, P, HID, transpose=True,
                                     queue_num=jj % 4)
                for half in range(2):
                    j = 2 * jj + half
                    w1_sb = wk2.tile([P, KC, NH1], BF16, tag="nhw1")
                    nc.sync.dma_start(
                        w1_sb[:],
                        dt["nh_w1"][j].rearrange("p (k f) -> p k f", k=KC))
                    w2_sb = wk2.tile([P, 2, NH2], BF16, tag="nhw2")
                    nc.sync.dma_start(
                        w2_sb[:],
                        dt["nh_w2"][j].rearrange("p (k f) -> p k f", k=2))
                    rhs = xpT[:, :, half * B:(half + 1) * B]
                    h1T = wk2.tile([P, 2, B], BF16, tag="h1T")
                    for mo in range(2):
                        ps = psB.tile([P, HID], F32, space="PSUM",
                                      tag="conv")
                        for k in range(KC):
                            nc.tensor.matmul(
                                ps[:, :B],
                                w1_sb[:, k, mo * P:(mo + 1) * P],
                                rhs[:, k, :], start=(k == 0),
                                stop=(k == KC - 1))
                        nc.scalar.activation(
                            h1T[:, mo, :], ps[:, :B], Relu,
                            bias=nhb1_sb[:, 2 * j + mo:2 * j + mo + 1])
                    ps2 = psB.tile([P, HID], F32, space="PSUM", tag="conv")
                    for k in range(2):
                        nc.tensor.matmul(ps2[:, :B], w2_sb[:, k, :],
                                         h1T[:, k, :], start=(k == 0),
                                         stop=(k == 1))
                    h2T = wk2.tile([P, B], BF16, tag="h2T")
                    nc.scalar.activation(h2T[:], ps2[:, :B], Relu,
                                         bias=nhb2_sb[:, j:j + 1])
                    ps3 = psA.tile([P, HID], F32, space="PSUM", tag="agg")
                    nc.tensor.matmul(ps3[:1, :B], nhw3_sb[:, j:j + 1],
                                     h2T[:], start=True, stop=True)
                    nc.scalar.activation(
                        nodeflat[0:1, j * B:(j + 1) * B], ps3[:1, :B],
                        Copy)
            nflat_d = nc.dram_tensor("nflat_d", [1, POS_PC * B], F32)
            nc.sync.dma_start(nflat_d[:], nodeflat[:])
            nodeT = sm.tile([POS_PC, B], F32, tag="nodeT")
            nc.sync.dma_start(
                nodeT[:],
                nflat_d[:].rearrange("o (j g) -> (o j) g", j=POS_PC))
            nodeS = sm.tile([POS_PC, B], F32, tag="nodeS")
            nc.vector.tensor_tensor(
                nodeS[:], nodeT[:],
                nhb3_sb[:].to_broadcast((POS_PC, B)), ADD)
            nc.sync.dma_start(out_node[:], nodeS[:])

    nc.compile()
    return nc


_PROG_CACHE = {}


def _get_program(TB):
    key = tuple(TB)
    if key not in _PROG_CACHE:
        _PROG_CACHE[key] = _build_program(TB)
    return _PROG_CACHE[key]


def kernel(**inputs):
    res = _run(inputs)
    return _assemble(res)


_LAST_RES = None


def _run(inputs, debug=False, trace=False):
    global _LAST_RES
    TB, maps = _host_prep(inputs)
    nc = _get_program(TB)
    res = run_bass_kernel_spmd(nc, maps, list(range(NC)), trace=trace)
    _LAST_RES = res
    if trace:
        print(f"HW exec time: {res.exec_time_ns} ns")
        print(f"mean exec time: {res.mean_exec_time_ns} ns "
              f"(max core {res.max_exec_time_core_id})")
    return res.results


def _assemble(results):
    full = np.empty((B, GOUT + NUM_NODES), np.float32)
    for c in range(NC):
        full[:, GOUT + POS_PC * c: GOUT + POS_PC * (c + 1)] = \
            results[c]["out_node"].T
        full[8 * c:8 * (c + 1), :GOUT] = results[c]["out_graph"][:, :8].T
    return full
